# revision 1
# baseline (speedup 1.0000x reference)
"""Causal self-attention (RoPE + QK-RMSNorm, GQA 16q/8kv) Trainium2 Bass kernel.

Sharding: 8 cores = 2 batch x 4 tensor-parallel. Core c handles batch b=c//4 and
q-heads [4*tp, 4*tp+4), kv-heads [2*tp, 2*tp+2) where tp=c%4. Each core returns a
partial (T, C) output = O_heads @ wo[rows of its heads]; host sums the 4 partials
per batch (the "all-reduce after c_proj").

Matmuls run in bf16 (fp32 PSUM accumulation); softmax row-sum normalization and
RMS statistics stay in fp32/fp32r.
"""
import sys
import math

sys.path.insert(0, "/opt/trn_rl_repo")

import numpy as np
import ml_dtypes
import concourse.bacc as bacc
import concourse.mybir as mybir
import concourse.tile as tile
from concourse.bass_utils import run_bass_kernel_spmd

P = 128
T = 2048
C = 2048
KO = C // P          # 16 contraction tiles
D = 128              # head dim
NQ = 4               # q heads per core
NK = 2               # kv heads per core
NF = NQ + NK         # 6 rope/rms feature blocks (4 q + 2 k)
FQ = NQ * D          # 512
FK = NK * D          # 256
TCH = 512            # phase-1 T-chunk
NCHUNK = T // TCH    # 4
SPAN = 512           # attention q-span
NSPAN = T // SPAN    # 4
KB = T // P          # 16 key blocks
SCALE = 1.0 / math.sqrt(D)
EPS = 1.1920929e-07

f32 = mybir.dt.float32
f32r = mybir.dt.float32r
bf16 = mybir.dt.bfloat16

AF = mybir.ActivationFunctionType


def build():
    nc = bacc.Bacc("TRN2", target_bir_lowering=False)
    xT = nc.dram_tensor("xT", (C, T), bf16, kind="ExternalInput")
    wq = nc.dram_tensor("wq", (C, FQ), bf16, kind="ExternalInput")
    wk = nc.dram_tensor("wk", (C, FK), bf16, kind="ExternalInput")
    wv = nc.dram_tensor("wv", (C, FK), bf16, kind="ExternalInput")
    wo = nc.dram_tensor("wo", (FQ, C), bf16, kind="ExternalInput")
    cc = nc.dram_tensor("cc", (P, T), f32, kind="ExternalInput")    # [cos; cos]
    ss = nc.dram_tensor("ss", (P, T), f32, kind="ExternalInput")    # [sin; -sin]
    maskT = nc.dram_tensor("maskT", (P, 4, SPAN), bf16, kind="ExternalInput")
    ident = nc.dram_tensor("ident", (P, P), bf16, kind="ExternalInput")
    y = nc.dram_tensor("y", (T, C), f32, kind="ExternalOutput")

    xT_r = xT.rearrange("(ko p) t -> p ko t", p=P)
    wq_r = wq.rearrange("(ko p) f -> p ko f", p=P)
    wk_r = wk.rearrange("(ko p) f -> p ko f", p=P)
    wv_r = wv.rearrange("(ko p) f -> p ko f", p=P)
    wo_r = wo.rearrange("(ko p) n -> p ko n", p=P)

    with tile.TileContext(nc) as tc:
        with tc.tile_pool(name="persist", bufs=1) as persist:
            # persistent across phases
            qk_rt = persist.tile([P, NF, T], bf16, tag="qk_rt")   # roped+normed qT/kT
            v_sb = persist.tile([P, KB, FK], bf16, tag="v_sb")    # V natural [t-part, kb, feat]
            cc_sb = persist.tile([P, T], f32, tag="cc_sb")
            ss_sb = persist.tile([P, T], f32, tag="ss_sb")
            id_sb = persist.tile([P, P], bf16, tag="id_sb")
            ones_col = persist.tile([P, 1], bf16, tag="ones_col")    # sums lhsT
            ones_row = persist.tile([1, P], f32r, tag="ones_row")    # bcast lhsT
            eps_sb = persist.tile([P, 1], f32, tag="eps_sb")
            zero_sb = persist.tile([1, 1], f32, tag="zero_sb")
            nc.vector.memset(zero_sb[:], 0.0)
            ones_f32 = persist.tile([P, 1], f32, tag="ones_f32")
            ones_row_f32 = persist.tile([1, P], f32, tag="ones_row_f32")
            nc.sync.dma_start(cc_sb[:], cc[:, :])
            nc.sync.dma_start(ss_sb[:], ss[:, :])
            nc.sync.dma_start(id_sb[:], ident[:, :])
            nc.vector.memset(eps_sb[:], EPS)
            nc.vector.memset(ones_f32[:], 1.0)
            nc.vector.memset(ones_row_f32[:], 1.0)
            nc.vector.tensor_copy(ones_col[:], ones_f32[:])
            nc.vector.tensor_copy(ones_row[:], ones_row_f32[:])

            # ------- Phase 1: QKV projections + RoPE + RMS norm + V transpose -------
            with (
                tc.tile_pool(name="ph1w", bufs=1) as wpool,
                tc.tile_pool(name="ph1x", bufs=2) as xpool,
                tc.tile_pool(name="ph1t", bufs=3) as tpool,
                tc.tile_pool(name="ph1ps", bufs=3, space="PSUM") as ps1,
                tc.tile_pool(name="ph1tr", bufs=1, space="PSUM") as pstr,
                tc.tile_pool(name="ph1ms", bufs=2, space="PSUM") as psms,
                tc.tile_pool(name="ph1rb", bufs=2, space="PSUM") as psrb,
            ):
                wq_sb = wpool.tile([P, KO, FQ], bf16, tag="wq_sb")
                wk_sb = wpool.tile([P, KO, FK], bf16, tag="wk_sb")
                wv_sb = wpool.tile([P, KO, FK], bf16, tag="wv_sb")
                nc.sync.dma_start(wq_sb[:], wq_r)
                nc.sync.dma_start(wk_sb[:], wk_r)
                nc.sync.dma_start(wv_sb[:], wv_r)

                for tch in range(NCHUNK):
                    t0 = tch * TCH
                    xt = xpool.tile([P, KO, TCH], bf16, tag="xt")
                    # per-ko DMAs so matmuls can start as slices land
                    for ko in range(KO):
                        nc.sync.dma_start(xt[:, ko, :], xT_r[:, ko, t0 : t0 + TCH])
                    # qT / kT feature blocks (4 q heads + 2 k heads)
                    sqs = []
                    for fb in range(NF):
                        if fb < NQ:
                            w_ap = wq_sb[:, :, fb * D : (fb + 1) * D]
                        else:
                            w_ap = wk_sb[:, :, (fb - NQ) * D : (fb - NQ + 1) * D]
                        pqk = ps1.tile([P, TCH], f32, tag="ps_qkv")
                        for ko in range(KO):
                            nc.tensor.matmul(
                                pqk[:], w_ap[:, ko], xt[:, ko, :],
                                start=(ko == 0), stop=(ko == KO - 1),
                            )
                        # rope: raw chunk + half-swapped chunk (fp32), write bf16
                        raw = tpool.tile([P, TCH], f32, tag="rope_raw")
                        nc.vector.tensor_copy(raw[:], pqk[:])
                        swp = tpool.tile([P, TCH], f32, tag="rope_swp")
                        nc.sync.dma_start(swp[0:64, :], raw[64:128, :])
                        nc.sync.dma_start(swp[64:128, :], raw[0:64, :])
                        tmpa = tpool.tile([P, TCH], f32, tag="rope_tmpa")
                        tmpb = tpool.tile([P, TCH], f32, tag="rope_tmpb")
                        seg = qk_rt[:, fb, t0 : t0 + TCH]
                        nc.vector.tensor_mul(tmpa[:], raw[:], cc_sb[:, t0 : t0 + TCH])
                        nc.vector.tensor_mul(tmpb[:], swp[:], ss_sb[:, t0 : t0 + TCH])
                        nc.vector.tensor_add(seg, tmpa[:], tmpb[:])
                        # RMS stats: sum of squares over head dim (partitions)
                        sq = tpool.tile([P, TCH], bf16, tag="sq")
                        nc.vector.tensor_mul(sq[:], seg, seg)
                        pms = psms.tile([1, TCH], f32, tag="ps_ms")
                        nc.tensor.matmul(pms[:], ones_col[:], sq[:], start=True, stop=True)
                        # rstd = exp(-0.5 * ln(ms/D + eps)) — both on ACT, off the PE path
                        lnms = tpool.tile([1, TCH], f32, tag="lnms")
                        nc.scalar.activation(
                            lnms[:], pms[:], AF.Ln, bias=eps_sb[0:1, :], scale=1.0 / D
                        )
                        rstd = tpool.tile([1, TCH], f32r, tag="rstd")
                        nc.scalar.activation(rstd[:], lnms[:], AF.Exp, scale=-0.5)
                        sqs.append((seg, rstd))
                    # RMS apply pass — bcast matmuls run a full block later so the
                    # ACT chain has drained and the PE never head-of-line blocks
                    for seg, rstd in sqs:
                        pb = psrb.tile([P, TCH], f32, tag="ps_b")
                        nc.tensor.matmul(pb[:], ones_row[:], rstd[:], start=True, stop=True)
                        nc.vector.tensor_mul(seg, seg, pb[:])
                    # vT blocks -> transpose -> V natural
                    for vfb in range(NK):
                        w_ap = wv_sb[:, :, vfb * D : (vfb + 1) * D]
                        pvt = ps1.tile([P, TCH], f32, tag="ps_qkv")
                        for ko in range(KO):
                            nc.tensor.matmul(
                                pvt[:], w_ap[:, ko], xt[:, ko, :],
                                start=(ko == 0), stop=(ko == KO - 1),
                            )
                        vt_sb = tpool.tile([P, TCH], bf16, tag="vt_sb")
                        nc.vector.tensor_copy(vt_sb[:], pvt[:])
                        for tb in range(TCH // P):
                            ptr = pstr.tile([P, P], bf16, tag="ps_tr")
                            nc.tensor.transpose(
                                ptr[:], vt_sb[:, tb * P : (tb + 1) * P], id_sb[:]
                            )
                            nc.vector.tensor_copy(
                                v_sb[:, tch * (TCH // P) + tb, vfb * D : (vfb + 1) * D],
                                ptr[:],
                            )

            # ---------------- Phase 3: attention + Phase 4: output projection ------------
            with (
                tc.tile_pool(name="ph3s", bufs=1) as p3s,
                tc.tile_pool(name="ph3t", bufs=6) as p3,
            ):
                ot_sb = p3s.tile([P, NQ, T], bf16, tag="ot_sb")
                mask_sb = p3s.tile([P, 4, SPAN], bf16, tag="mask_sb")
                wo_sb = p3s.tile([P, NQ, C], bf16, tag="wo_sb")
                nc.sync.dma_start(mask_sb[:], maskT[:, :, :])
                nc.sync.dma_start(wo_sb[:], wo_r)

                with (
                    tc.tile_pool(name="ph3ps", bufs=3, space="PSUM") as ps3,
                    tc.tile_pool(name="ph3ot", bufs=2, space="PSUM") as psot,
                    tc.tile_pool(name="ph3m", bufs=1, space="PSUM") as psm,
                ):
                  for s in range(NSPAN):
                    q0 = s * SPAN
                    nkb = 4 * s + 4
                    for h in range(NQ):
                        j = h // 2
                        ot_ps = psot.tile([P, SPAN], f32, tag="ot_ps")
                        sum_ps = psot.tile([1, SPAN], f32, tag="sum_ps")
                        q_ap = qk_rt[:, h, q0 : q0 + SPAN]
                        for kb in range(nkb):
                            st_ps = ps3.tile([P, SPAN], f32, tag="st_ps")
                            nc.tensor.matmul(
                                st_ps[:],
                                qk_rt[:, NQ + j, kb * P : (kb + 1) * P],
                                q_ap,
                                start=True, stop=True,
                            )
                            pt = p3.tile([P, SPAN], bf16, tag="pt")
                            nc.scalar.activation(pt[:], st_ps[:], AF.Exp, scale=SCALE)
                            if kb >= 4 * s:
                                nc.vector.tensor_mul(
                                    pt[:], pt[:], mask_sb[:, kb - 4 * s, :]
                                )
                            nc.tensor.matmul(
                                ot_ps[:],
                                v_sb[:, kb, j * D : (j + 1) * D],
                                pt[:],
                                start=(kb == 0), stop=(kb == nkb - 1),
                                skip_group_check=True,
                            )
                            nc.tensor.matmul(
                                sum_ps[:],
                                ones_col[:],
                                pt[:],
                                start=(kb == 0), stop=(kb == nkb - 1),
                                skip_group_check=True,
                            )
                        # normalization: 1/sums = exp(-ln(sums)) on ACT, then bcast
                        lns = p3.tile([1, SPAN], f32, tag="lns")
                        nc.scalar.activation(lns[:], sum_ps[:], AF.Ln)
                        rec = p3.tile([1, SPAN], f32r, tag="rec")
                        nc.scalar.activation(rec[:], lns[:], AF.Exp, scale=-1.0)
                        bc_ps = psm.tile([P, SPAN], f32, tag="m512")
                        nc.tensor.matmul(bc_ps[:], ones_row[:], rec[:], start=True, stop=True)
                        bc_sb = p3.tile([P, SPAN], f32, tag="bc_sb")
                        nc.scalar.activation(bc_sb[:], bc_ps[:], AF.Copy)
                        nc.vector.tensor_mul(
                            ot_sb[:, h, q0 : q0 + SPAN], ot_ps[:], bc_sb[:]
                        )

                    # output projection for the T-blocks of this span
                    for tb in range(4 * s, 4 * s + 4):
                        for nch in range(C // 512):
                            yps = psm.tile([P, 512], f32, tag="m512")
                            for h in range(NQ):
                                nc.tensor.matmul(
                                    yps[:],
                                    ot_sb[:, h, tb * P : (tb + 1) * P],
                                    wo_sb[:, h, nch * 512 : (nch + 1) * 512],
                                    start=(h == 0), stop=(h == NQ - 1),
                                )
                            ysb = p3.tile([P, 512], f32, tag="ysb")
                            nc.vector.tensor_copy(ysb[:], yps[:])
                            nc.sync.dma_start(
                                y[tb * P : (tb + 1) * P, nch * 512 : (nch + 1) * 512],
                                ysb[:],
                            )
    nc.compile()
    return nc


_NC_CACHE = None


def _get_nc():
    global _NC_CACHE
    if _NC_CACHE is None:
        _NC_CACHE = build()
    return _NC_CACHE


def _host_inputs(x, cos, sin, wq, wk, wv, wo):
    """Build the 8 per-core input maps."""
    bft = ml_dtypes.bfloat16
    cosT = np.ascontiguousarray(cos[0, :, 0, :].T).astype(np.float32)  # (64, T)
    sinT = np.ascontiguousarray(sin[0, :, 0, :].T).astype(np.float32)
    cc = np.concatenate([cosT, cosT], axis=0)          # (128, T)
    ss = np.concatenate([sinT, -sinT], axis=0)
    # maskT[r][k, q] = 1 if q >= 128*r + k  (within a 512-q span, k-block offset r)
    qidx = np.arange(SPAN)[None, None, :]
    kidx = np.arange(P)[:, None, None]
    ridx = np.arange(4)[None, :, None]
    maskT = (qidx >= P * ridx + kidx).astype(bft)  # (128, 4, 512)
    ident = np.eye(P, dtype=np.float32).astype(bft)

    xTs = [np.ascontiguousarray(x[b].T).astype(bft) for b in range(2)]
    wq16 = wq.astype(bft)
    wk16 = wk.astype(bft)
    wv16 = wv.astype(bft)
    wo16 = wo.astype(bft)
    in_maps = []
    for c in range(8):
        b, tp = divmod(c, 4)
        in_maps.append(
            {
                "xT": xTs[b],
                "wq": np.ascontiguousarray(wq16[:, tp * FQ : (tp + 1) * FQ]),
                "wk": np.ascontiguousarray(wk16[:, tp * FK : (tp + 1) * FK]),
                "wv": np.ascontiguousarray(wv16[:, tp * FK : (tp + 1) * FK]),
                "wo": np.ascontiguousarray(wo16[tp * FQ : (tp + 1) * FQ, :]),
                "cc": cc,
                "ss": ss,
                "maskT": maskT,
                "ident": ident,
            }
        )
    return in_maps


def kernel(x, cos, sin, wq, wk, wv, wo, trace=False):
    x = np.asarray(x, dtype=np.float32)
    cos = np.asarray(cos, dtype=np.float32)
    sin = np.asarray(sin, dtype=np.float32)
    wq = np.asarray(wq, dtype=np.float32)
    wk = np.asarray(wk, dtype=np.float32)
    wv = np.asarray(wv, dtype=np.float32)
    wo = np.asarray(wo, dtype=np.float32)

    nc = _get_nc()
    in_maps = _host_inputs(x, cos, sin, wq, wk, wv, wo)
    res = run_bass_kernel_spmd(nc, in_maps, core_ids=list(range(8)), trace=trace)
    out = np.zeros((2, T, C), dtype=np.float32)
    for c in range(8):
        b = c // 4
        out[b] += res.results[c]["y"]
    if trace:
        return out, res
    return out



# revision 7
# speedup vs baseline: 1.2476x; 1.2476x over previous
"""Causal self-attention (RoPE + QK-RMSNorm, GQA 16q/8kv) Trainium2 Bass kernel.

Sharding: 8 cores = 2 batch x 4 tensor-parallel. Core c handles batch b=c//4 and
q-heads [4*tp, 4*tp+4), kv-heads [2*tp, 2*tp+2) where tp=c%4. Each core returns a
partial (T, C) output = O_heads @ wo[rows of its heads]; host sums the 4 partials
per batch (the "all-reduce after c_proj").

Perf notes vs the first working version:
- The scalar engine only ever runs Sqrt (phase 1) and Exp (attention) plus Copy,
  so there is no per-iteration activation-table reload.
- The PE stream is ordered so every cross-engine dependency has >1us of queued
  PE work in front of it: RMS stats matmuls are deferred one feature block,
  attention AV/sum matmuls run one kb-pair behind the score matmuls, and the
  output projection of span s-1 is interleaved into span s's loop as filler.
  This keeps the tensor engine continuously busy, which also keeps it at the
  2.4 GHz p-state (it drops to 1.2 GHz for 3us after every idle gap).
- V is produced directly in [t, feat] layout (x-block as stationary operand),
  removing the PE transposes.
- Softmax normalization: row sums via ones-column matmuls (PSUM), reciprocal on
  DVE (reciprocal_approx_fast), broadcast across partitions on GpSimd.
- Causal diagonal blocks are trimmed: score/exp/AV/sum only touch q-columns
  that can be live for that key block.
"""
import sys
import math

sys.path.insert(0, "/opt/trn_rl_repo")

import numpy as np
import ml_dtypes
import concourse.bacc as bacc
import concourse.mybir as mybir
import concourse.tile as tile
from concourse.bass_utils import run_bass_kernel_spmd

P = 128
T = 2048
C = 2048
KO = C // P          # 16 contraction tiles
D = 128              # head dim
NQ = 4               # q heads per core
NK = 2               # kv heads per core
NF = NQ + NK         # 6 rope/rms feature blocks (2 k + 4 q)
FQ = NQ * D          # 512
FK = NK * D          # 256
TCH = 512            # phase-1 T-chunk
NCHUNK = T // TCH    # 4
SPAN = 512           # attention q-span
NSPAN = T // SPAN    # 4
KB = T // P          # 16 key blocks
SCALE = 1.0 / math.sqrt(D)
EPS = 1.1920929e-07

f32 = mybir.dt.float32
bf16 = mybir.dt.bfloat16

AF = mybir.ActivationFunctionType

# feature-block order in qk_rt: k0, k1, q0..q3 (K first so the last chunk's
# K is ready the moment attention starts)
FB_K = [0, 1]

def fb_of_q(h):
    return NK + h


def build():
    nc = bacc.Bacc("TRN2", target_bir_lowering=False)
    xT = nc.dram_tensor("xT", (C, T), bf16, kind="ExternalInput")
    wq = nc.dram_tensor("wq", (C, FQ), bf16, kind="ExternalInput")
    wk = nc.dram_tensor("wk", (C, FK), bf16, kind="ExternalInput")
    wv = nc.dram_tensor("wv", (C, FK), bf16, kind="ExternalInput")
    wo = nc.dram_tensor("wo", (FQ, C), bf16, kind="ExternalInput")
    cc = nc.dram_tensor("cc", (P, T), f32, kind="ExternalInput")    # [cos; cos]
    ss = nc.dram_tensor("ss", (P, T), f32, kind="ExternalInput")    # [sin; -sin]
    mask = nc.dram_tensor("mask", (P, P), bf16, kind="ExternalInput")  # tri: c>=p
    y = nc.dram_tensor("y", (T, C), bf16, kind="ExternalOutput")

    xT_r = xT.rearrange("(ko p) t -> p ko t", p=P)
    wq_r = wq.rearrange("(ko p) f -> p ko f", p=P)
    wk_r = wk.rearrange("(ko p) f -> p ko f", p=P)
    wv_r = wv.rearrange("(ko p) f -> p ko f", p=P)
    wo_r = wo.rearrange("(ko p) n -> p ko n", p=P)

    with tile.TileContext(nc) as tc:
        with tc.tile_pool(name="persist", bufs=1) as persist:
            qk_rt = persist.tile([P, NF, T], bf16, tag="qk_rt")   # roped+normed kT/qT
            v_sb = persist.tile([P, KB, FK], bf16, tag="v_sb")    # V natural [t-part, kb, feat]
            cc_sb = persist.tile([P, T], f32, tag="cc_sb")
            ss_sb = persist.tile([P, T], f32, tag="ss_sb")
            mask_sb = persist.tile([P, P], bf16, tag="mask_sb")
            ones_col = persist.tile([P, 1], bf16, tag="ones_col")
            ones_f32 = persist.tile([P, 1], f32, tag="ones_f32")
            ot_sb = persist.tile([P, NQ, T], bf16, tag="ot_sb")
            wo_sb = persist.tile([P, NQ, C], bf16, tag="wo_sb")

            eps_sb = persist.tile([1, 1], f32, tag="eps_sb")
            nc.vector.memset(eps_sb[:], EPS)
            nc.vector.memset(ones_f32[:], 1.0)
            nc.vector.tensor_copy(ones_col[:], ones_f32[:])
            nc.sync.dma_start(mask_sb[:], mask[:, :])

            # ---------------- Phase 1: QKV + RoPE + RMS norm ----------------
            with (
                tc.tile_pool(name="ph1w", bufs=1) as wpool,
                tc.tile_pool(name="ph1x", bufs=2) as xpool,
                tc.tile_pool(name="ph1t", bufs=3) as tpool,
                tc.tile_pool(name="ph1s", bufs=3) as spool,
                tc.tile_pool(name="ph1ps", bufs=2, space="PSUM") as psqk,
                tc.tile_pool(name="ph1pv", bufs=2, space="PSUM") as psv,
                tc.tile_pool(name="ph1ms", bufs=2, space="PSUM") as psms,
            ):
                wq_sb = wpool.tile([P, KO, FQ], bf16, tag="wq_sb")
                wk_sb = wpool.tile([P, KO, FK], bf16, tag="wk_sb")
                wv_sb = wpool.tile([P, KO, FK], bf16, tag="wv_sb")
                # priority-ordered weight DMAs: k slices first (fb order), then
                # q slices, then v (needed at end of each chunk)
                for j in range(NK):
                    nc.sync.dma_start(
                        wk_sb[:, :, j * D : (j + 1) * D], wk_r[:, :, j * D : (j + 1) * D]
                    )
                nc.sync.dma_start(cc_sb[:, 0:TCH], cc[:, 0:TCH])
                nc.sync.dma_start(ss_sb[:, 0:TCH], ss[:, 0:TCH])
                for h in range(NQ):
                    nc.sync.dma_start(
                        wq_sb[:, :, h * D : (h + 1) * D], wq_r[:, :, h * D : (h + 1) * D]
                    )
                nc.sync.dma_start(wv_sb[:], wv_r)

                # weight source AP per feature block (fb order: k0 k1 q0..q3)
                def w_ap(fb):
                    if fb < NK:
                        return wk_sb[:, :, fb * D : (fb + 1) * D]
                    h = fb - NK
                    return wq_sb[:, :, h * D : (h + 1) * D]

                xts = [None] * NCHUNK

                def dma_chunk_part(tch, part):
                    # split the 16 per-ko DMAs into 4 batches so latency-
                    # critical swap DMAs aren't queued behind them on sync
                    t0 = tch * TCH
                    if part == 0:
                        xts[tch] = xpool.tile([P, KO, TCH], bf16, tag="xt", name="xt")
                    xt = xts[tch]
                    for ko in range(4 * part, 4 * part + 4):
                        nc.sync.dma_start(xt[:, ko, :], xT_r[:, ko, t0 : t0 + TCH])
                    if part == 3 and tch > 0:
                        nc.sync.dma_start(cc_sb[:, t0 : t0 + TCH], cc[:, t0 : t0 + TCH])
                        nc.sync.dma_start(ss_sb[:, t0 : t0 + TCH], ss[:, t0 : t0 + TCH])

                for part in range(4):
                    dma_chunk_part(0, part)

                # deferred work queue: closures emitting the rope/stats/apply
                # chain pieces, run 1-2 fb slots after the matmuls they depend on
                for tch in range(NCHUNK):
                    t0 = tch * TCH
                    xt = xts[tch]
                    stats_q = []   # (emit_stats, emit_finish) per fb
                    finish_q = []

                    def emit_fb(fb, tch=tch, t0=t0, xt=xt):
                        pqk = psqk.tile([P, TCH], f32, tag="pqk")
                        for ko in range(KO):
                            nc.tensor.matmul(
                                pqk[:], w_ap(fb)[:, ko], xt[:, ko, :],
                                start=(ko == 0), stop=(ko == KO - 1),
                            )
                        # rope chain (ACT/DMA/DVE; runs while PE does next fb)
                        raw = tpool.tile([P, TCH], f32, tag="raw")
                        nc.scalar.copy(raw[:], pqk[:])
                        swp = tpool.tile([P, TCH], f32, tag="swp")
                        nc.sync.dma_start(swp[0:64, :], raw[64:128, :])
                        nc.sync.dma_start(swp[64:128, :], raw[0:64, :])
                        seg = qk_rt[:, fb, t0 : t0 + TCH]
                        tmpa = tpool.tile([P, TCH], f32, tag="tmpa")
                        nc.vector.tensor_mul(tmpa[:], pqk[:], cc_sb[:, t0 : t0 + TCH])
                        tmpb = tpool.tile([P, TCH], f32, tag="tmpb")
                        nc.vector.tensor_mul(tmpb[:], swp[:], ss_sb[:, t0 : t0 + TCH])
                        nc.vector.tensor_add(seg, tmpa[:], tmpb[:])
                        sq = spool.tile([P, TCH], bf16, tag="sq")
                        nc.vector.tensor_mul(sq[:], seg, seg)

                        def emit_stats(fb=fb, sq=sq):
                            pms = psms.tile([1, TCH], f32, tag="pms")
                            nc.tensor.matmul(
                                pms[:], ones_col[:], sq[:], start=True, stop=True
                            )
                            # rms = sqrt(ms/D + eps) on ACT (same table all phase)
                            rms = spool.tile([1, TCH], f32, tag="rms")
                            nc.scalar.activation(
                                rms[:], pms[:], AF.Sqrt, bias=eps_sb[0:1, :],
                                scale=1.0 / D,
                            )
                            return rms

                        def emit_finish(rms, fb=fb, seg=seg):
                            rstd = spool.tile([1, TCH], f32, tag="rstd")
                            nc.vector.reciprocal_approx_fast(rstd[:], rms[:])
                            rb = tpool.tile([P, TCH], f32, tag="rb")
                            nc.gpsimd.partition_broadcast(rb[:], rstd[:])
                            nc.vector.tensor_mul(seg, seg, rb[:])

                        stats_q.append(emit_stats)
                        finish_q.append(emit_finish)

                    # V blocks, natural layout: stationary = x block, moving = wv
                    def emit_v(tb, tch=tch, xt=xt):
                        pv = psv.tile([P, FK], f32, tag="pv")
                        for ko in range(KO):
                            nc.tensor.matmul(
                                pv[:], xt[:, ko, tb * P : (tb + 1) * P], wv_sb[:, ko, :],
                                start=(ko == 0), stop=(ko == KO - 1),
                            )
                        nc.vector.tensor_copy(v_sb[:, tch * (TCH // P) + tb, :], pv[:])

                    # PE emission order for this chunk: stats matmul for fb is
                    # emitted 2 fb-slots later, the rstd/apply chain 4 slots
                    # later, so the PE never waits on the DVE/ACT chains.
                    rms_tiles = [None] * NF
                    for fb in range(NF):
                        emit_fb(fb)
                        if fb >= 2 and tch + 1 < NCHUNK:
                            dma_chunk_part(tch + 1, fb - 2)
                        if fb >= 2:
                            rms_tiles[fb - 2] = stats_q[fb - 2]()
                        if fb >= 4:
                            finish_q[fb - 4](rms_tiles[fb - 4])
                    for tb in range(TCH // P):
                        emit_v(tb)
                        if tb < 2:
                            rms_tiles[NF - 2 + tb] = stats_q[NF - 2 + tb]()
                        finish_q[NF - 4 + tb](rms_tiles[NF - 4 + tb])

                nc.sync.dma_start(wo_sb[:], wo_r)

            # ---------------- Phase 2: attention + output projection ----------------
            with (
                tc.tile_pool(name="at_pt", bufs=4) as ptpool,
                tc.tile_pool(name="at_sb", bufs=3) as asb,
                tc.tile_pool(name="at_rb", bufs=2) as rbpool,
                tc.tile_pool(name="at_y", bufs=3) as ypool,
                tc.tile_pool(name="at_st", bufs=1, space="PSUM") as psst,
                tc.tile_pool(name="at_ot", bufs=2, space="PSUM") as psot,
                tc.tile_pool(name="at_sm", bufs=2, space="PSUM") as pssm,
                tc.tile_pool(name="at_yp", bufs=2, space="PSUM") as psy,
            ):
                pending_proj = []

                def emit_proj_group():
                    tb, nch = pending_proj.pop(0)
                    yps = psy.tile([P, 512], f32, tag="yps")
                    for h in range(NQ):
                        nc.tensor.matmul(
                            yps[:],
                            ot_sb[:, h, tb * P : (tb + 1) * P],
                            wo_sb[:, h, nch * 512 : (nch + 1) * 512],
                            start=(h == 0), stop=(h == NQ - 1),
                        )
                    ysb = ypool.tile([P, 512], bf16, tag="ysb")
                    nc.vector.tensor_copy(ysb[:], yps[:])
                    nc.sync.dma_start(
                        y[tb * P : (tb + 1) * P, nch * 512 : (nch + 1) * 512], ysb[:]
                    )

                for s in range(NSPAN):
                    q0 = s * SPAN
                    nkb = 4 * (s + 1)
                    npair = nkb // 2

                    def off_of(kb, s=s):
                        return P * (kb - 4 * s) if kb >= 4 * s else 0

                    for h in range(NQ):
                        j = h // 2
                        q_ap = qk_rt[:, fb_of_q(h), q0 : q0 + SPAN]
                        ot_ps = psot.tile([P, SPAN], f32, tag="ot_ps")
                        sum_ps = pssm.tile([1, SPAN], f32, tag="sum_ps")

                        def emit_avsum(kb, pt_ap, h=h, j=j, nkb=nkb,
                                       ot_ps=ot_ps, sum_ps=sum_ps):
                            off = off_of(kb)
                            nc.tensor.matmul(
                                ot_ps[:, off:], v_sb[:, kb, j * D : (j + 1) * D],
                                pt_ap,
                                start=(kb == 0), stop=(kb == nkb - 1),
                                skip_group_check=True,
                            )
                            nc.tensor.matmul(
                                sum_ps[:, off:], ones_col[:], pt_ap,
                                start=(kb == 0), stop=(kb == nkb - 1),
                                skip_group_check=True,
                            )

                        prev = None  # (kb_a, pt_a_ap, kb_b, pt_b_ap)
                        for i in range(npair):
                            ka, kbb = 2 * i, 2 * i + 1
                            offa, offb = off_of(ka), off_of(kbb)
                            diag = kbb >= 4 * s
                            st2 = psst.tile([P, 2, SPAN], f32, tag="st2")
                            nc.tensor.matmul(
                                st2[:, 0, offa:],
                                qk_rt[:, j, ka * P : (ka + 1) * P],
                                q_ap[:, offa:],
                                start=True, stop=True,
                            )
                            nc.tensor.matmul(
                                st2[:, 1, offb:],
                                qk_rt[:, j, kbb * P : (kbb + 1) * P],
                                q_ap[:, offb:],
                                start=True, stop=True,
                            )
                            # PE filler while exp runs: previous pair's AV/sums
                            if prev is not None:
                                emit_avsum(prev[0], prev[1])
                                emit_avsum(prev[2], prev[3])
                            if pending_proj:
                                emit_proj_group()
                            # exp (+ mask on diagonal blocks)
                            pt2 = ptpool.tile([P, 2, SPAN], bf16, tag="pt2")
                            if diag:
                                nc.scalar.activation(
                                    pt2[:, 0, offa:], st2[:, 0, offa:], AF.Exp,
                                    scale=SCALE,
                                )
                                nc.scalar.activation(
                                    pt2[:, 1, offb:], st2[:, 1, offb:], AF.Exp,
                                    scale=SCALE,
                                )
                                if ka >= 4 * s:
                                    nc.vector.tensor_mul(
                                        pt2[:, 0, offa : offa + P],
                                        pt2[:, 0, offa : offa + P],
                                        mask_sb[:],
                                    )
                                nc.vector.tensor_mul(
                                    pt2[:, 1, offb : offb + P],
                                    pt2[:, 1, offb : offb + P],
                                    mask_sb[:],
                                )
                            else:
                                nc.scalar.activation(
                                    pt2[:, :, :], st2[:, :, :], AF.Exp, scale=SCALE
                                )
                            prev = (ka, pt2[:, 0, offa:], kbb, pt2[:, 1, offb:])
                        emit_avsum(prev[0], prev[1])
                        emit_avsum(prev[2], prev[3])

                        # normalization tail: 1/sums on DVE, bcast on GpSimd
                        rec = asb.tile([1, SPAN], f32, tag="rec")
                        nc.vector.reciprocal_approx_fast(rec[:], sum_ps[:])
                        rb = rbpool.tile([P, SPAN], f32, tag="rb2")
                        nc.gpsimd.partition_broadcast(rb[:], rec[:])
                        nc.vector.tensor_mul(
                            ot_sb[:, h, q0 : q0 + SPAN], ot_ps[:], rb[:]
                        )

                    # queue this span's projection; emitted inside span s+1's
                    # loop as PE filler (span 3's drains below)
                    for tb in range(4 * s, 4 * s + 4):
                        for nch in range(C // 512):
                            pending_proj.append((tb, nch))

                while pending_proj:
                    emit_proj_group()
    nc.compile()
    return nc


_NC_CACHE = None


def _get_nc():
    global _NC_CACHE
    if _NC_CACHE is None:
        _NC_CACHE = build()
    return _NC_CACHE


def _host_inputs(x, cos, sin, wq, wk, wv, wo):
    """Build the 8 per-core input maps."""
    bft = ml_dtypes.bfloat16
    cosT = np.ascontiguousarray(cos[0, :, 0, :].T).astype(np.float32)  # (64, T)
    sinT = np.ascontiguousarray(sin[0, :, 0, :].T).astype(np.float32)
    cc = np.concatenate([cosT, cosT], axis=0)          # (128, T)
    ss = np.concatenate([sinT, -sinT], axis=0)
    # mask[p, c] = 1 if c >= p (upper-tri incl diag): within a diagonal
    # 128-block, q-offset c sees key-offset p iff c >= p
    mask = (np.arange(P)[None, :] >= np.arange(P)[:, None]).astype(bft)

    xTs = [np.ascontiguousarray(x[b].T).astype(bft) for b in range(2)]
    wq16 = wq.astype(bft)
    wk16 = wk.astype(bft)
    wv16 = wv.astype(bft)
    wo16 = wo.astype(bft)
    in_maps = []
    for c in range(8):
        b, tp = divmod(c, 4)
        in_maps.append(
            {
                "xT": xTs[b],
                "wq": np.ascontiguousarray(wq16[:, tp * FQ : (tp + 1) * FQ]),
                "wk": np.ascontiguousarray(wk16[:, tp * FK : (tp + 1) * FK]),
                "wv": np.ascontiguousarray(wv16[:, tp * FK : (tp + 1) * FK]),
                "wo": np.ascontiguousarray(wo16[tp * FQ : (tp + 1) * FQ, :]),
                "cc": cc,
                "ss": ss,
                "mask": mask,
            }
        )
    return in_maps


def kernel(x, cos, sin, wq, wk, wv, wo, trace=False):
    x = np.asarray(x, dtype=np.float32)
    cos = np.asarray(cos, dtype=np.float32)
    sin = np.asarray(sin, dtype=np.float32)
    wq = np.asarray(wq, dtype=np.float32)
    wk = np.asarray(wk, dtype=np.float32)
    wv = np.asarray(wv, dtype=np.float32)
    wo = np.asarray(wo, dtype=np.float32)

    nc = _get_nc()
    in_maps = _host_inputs(x, cos, sin, wq, wk, wv, wo)
    res = run_bass_kernel_spmd(nc, in_maps, core_ids=list(range(8)), trace=trace)
    out = np.zeros((2, T, C), dtype=np.float32)
    for c in range(8):
        b = c // 4
        out[b] += res.results[c]["y"].astype(np.float32)
    if trace:
        return out, res
    return out


# revision 15
# speedup vs baseline: 1.3306x; 1.0665x over previous
"""Causal self-attention (RoPE + QK-RMSNorm, GQA 16q/8kv) Trainium2 Bass kernel.

Sharding: 8 cores = 2 batch x 4 tensor-parallel. Core c handles batch b=c//4 and
q-heads [4*tp, 4*tp+4), kv-heads [2*tp, 2*tp+2) where tp=c%4. Each core returns a
partial (T, C) output = O_heads @ wo[rows of its heads]; host sums the 4 partials
per batch (the "all-reduce after c_proj").

Perf notes vs the first working version:
- The scalar engine only ever runs Sqrt (phase 1) and Exp (attention) plus Copy,
  so there is no per-iteration activation-table reload.
- The PE stream is ordered so every cross-engine dependency has >1us of queued
  PE work in front of it: RMS stats matmuls are deferred one feature block,
  attention AV/sum matmuls run one kb-pair behind the score matmuls, and the
  output projection of span s-1 is interleaved into span s's loop as filler.
  This keeps the tensor engine continuously busy, which also keeps it at the
  2.4 GHz p-state (it drops to 1.2 GHz for 3us after every idle gap).
- V is produced directly in [t, feat] layout (x-block as stationary operand),
  removing the PE transposes.
- Softmax normalization: row sums via ones-column matmuls (PSUM), reciprocal on
  DVE (reciprocal_approx_fast), broadcast across partitions on GpSimd.
- Causal diagonal blocks are trimmed: score/exp/AV/sum only touch q-columns
  that can be live for that key block.
"""
import sys
import math

sys.path.insert(0, "/opt/trn_rl_repo")

import numpy as np
import ml_dtypes
import concourse.bacc as bacc
import concourse.mybir as mybir
import concourse.tile as tile
from concourse.bass_utils import run_bass_kernel_spmd

P = 128
T = 2048
C = 2048
KO = C // P          # 16 contraction tiles
D = 128              # head dim
NQ = 4               # q heads per core
NK = 2               # kv heads per core
NF = NQ + NK         # 6 rope/rms feature blocks (2 k + 4 q)
FQ = NQ * D          # 512
FK = NK * D          # 256
TCH = 512            # phase-1 T-chunk
NCHUNK = T // TCH    # 4
SPAN = 512           # attention q-span
NSPAN = T // SPAN    # 4
KB = T // P          # 16 key blocks
SCALE = 1.0 / math.sqrt(D)
EPS = 1.1920929e-07

f32 = mybir.dt.float32
bf16 = mybir.dt.bfloat16

AF = mybir.ActivationFunctionType

# feature-block order in qk_rt: k0, k1, q0..q3 (K first so the last chunk's
# K is ready the moment attention starts)
FB_K = [0, 1]

def fb_of_q(h):
    return NK + h


def build():
    nc = bacc.Bacc("TRN2", target_bir_lowering=False)
    xT = nc.dram_tensor("xT", (C, T), bf16, kind="ExternalInput")
    wq = nc.dram_tensor("wq", (C, FQ), bf16, kind="ExternalInput")
    wk = nc.dram_tensor("wk", (C, FK), bf16, kind="ExternalInput")
    wv = nc.dram_tensor("wv", (C, FK), bf16, kind="ExternalInput")
    wo = nc.dram_tensor("wo", (FQ, C), bf16, kind="ExternalInput")
    cc = nc.dram_tensor("cc", (P, T), f32, kind="ExternalInput")    # [cos; cos]
    ss = nc.dram_tensor("ss", (P, T), f32, kind="ExternalInput")    # [sin; -sin]
    mask = nc.dram_tensor("mask", (P, P), bf16, kind="ExternalInput")  # tri: c>=p
    y = nc.dram_tensor("y", (T, C), bf16, kind="ExternalOutput")

    xT_r = xT.rearrange("(ko p) t -> p ko t", p=P)
    wq_r = wq.rearrange("(ko p) f -> p ko f", p=P)
    wk_r = wk.rearrange("(ko p) f -> p ko f", p=P)
    wv_r = wv.rearrange("(ko p) f -> p ko f", p=P)
    wo_r = wo.rearrange("(ko p) n -> p ko n", p=P)

    with tile.TileContext(nc) as tc:
        with tc.tile_pool(name="persist", bufs=1) as persist:
            qk_rt = persist.tile([P, NF, T], bf16, tag="qk_rt")   # roped+normed kT/qT
            v_sb = persist.tile([P, KB, FK], bf16, tag="v_sb")    # V natural [t-part, kb, feat]
            cc_sb = persist.tile([P, T], f32, tag="cc_sb")
            ss_sb = persist.tile([P, T], f32, tag="ss_sb")
            mask_sb = persist.tile([P, P], bf16, tag="mask_sb")
            ones_col = persist.tile([P, 1], bf16, tag="ones_col")
            ones_f32 = persist.tile([P, 1], f32, tag="ones_f32")
            ot_sb = persist.tile([P, NQ, T], bf16, tag="ot_sb")
            wo_sb = persist.tile([P, NQ, C], bf16, tag="wo_sb")

            eps_sb = persist.tile([1, 1], f32, tag="eps_sb")
            nc.vector.memset(eps_sb[:], EPS)
            nc.vector.memset(ones_f32[:], 1.0)
            nc.vector.tensor_copy(ones_col[:], ones_f32[:])
            nc.sync.dma_start(mask_sb[:], mask[:, :])

            # ---------------- Phase 1: QKV + RoPE + RMS norm ----------------
            with (
                tc.tile_pool(name="ph1w", bufs=1) as wpool,
                tc.tile_pool(name="ph1x", bufs=2) as xpool,
                tc.tile_pool(name="ph1t", bufs=3) as tpool,
                tc.tile_pool(name="ph1s", bufs=3) as spool,
                tc.tile_pool(name="ph1ps", bufs=2, space="PSUM") as psqk,
                tc.tile_pool(name="ph1pv", bufs=2, space="PSUM") as psv,
                tc.tile_pool(name="ph1ms", bufs=2, space="PSUM") as psms,
            ):
                wq_sb = wpool.tile([P, KO, FQ], bf16, tag="wq_sb")
                wk_sb = wpool.tile([P, KO, FK], bf16, tag="wk_sb")
                wv_sb = wpool.tile([P, KO, FK], bf16, tag="wv_sb")

                # weight source AP per feature block (fb order: k0 k1 q0..q3)
                def w_ap(fb):
                    if fb < NK:
                        return wk_sb[:, :, fb * D : (fb + 1) * D]
                    h = fb - NK
                    return wq_sb[:, :, h * D : (h + 1) * D]

                xts = [None] * NCHUNK

                def dma_chunk_part(tch, part):
                    # split the 16 per-ko DMAs into 4 batches so latency-
                    # critical swap DMAs aren't queued behind them on sync
                    t0 = tch * TCH
                    if part == 0:
                        xts[tch] = xpool.tile([P, KO, TCH], bf16, tag="xt", name="xt")
                    xt = xts[tch]
                    for ko in range(4 * part, 4 * part + 4):
                        nc.sync.dma_start(xt[:, ko, :], xT_r[:, ko, t0 : t0 + TCH])

                # startup DMA priority order: the very first matmuls need only
                # wk slice j=0 and xt chunk-0, so emit those first and push the
                # q/v weights (needed 10+ us later) behind them
                nc.sync.dma_start(wk_sb[:, :, 0:D], wk_r[:, :, 0:D])
                dma_chunk_part(0, 0)
                dma_chunk_part(0, 1)
                nc.sync.dma_start(wk_sb[:, :, D : 2 * D], wk_r[:, :, D : 2 * D])
                dma_chunk_part(0, 2)
                dma_chunk_part(0, 3)
                nc.sync.dma_start(cc_sb[:, 0:TCH], cc[:, 0:TCH])
                nc.sync.dma_start(ss_sb[:, 0:TCH], ss[:, 0:TCH])
                for h in range(NQ):
                    nc.sync.dma_start(
                        wq_sb[:, :, h * D : (h + 1) * D], wq_r[:, :, h * D : (h + 1) * D]
                    )
                nc.sync.dma_start(wv_sb[:], wv_r)

                # deferred work queue: closures emitting the rope/stats/apply
                # chain pieces, run 1-2 fb slots after the matmuls they depend on
                for tch in range(NCHUNK):
                    t0 = tch * TCH
                    xt = xts[tch]
                    stats_q = []   # (emit_stats, emit_finish) per fb
                    finish_q = []

                    def emit_fb(fb, tch=tch, t0=t0, xt=xt):
                        pqk = psqk.tile([P, TCH], f32, tag="pqk")
                        for ko in range(KO):
                            nc.tensor.matmul(
                                pqk[:], w_ap(fb)[:, ko], xt[:, ko, :],
                                start=(ko == 0), stop=(ko == KO - 1),
                            )
                        # rope chain (ACT/DMA/DVE; runs while PE does next fb)
                        raw = tpool.tile([P, TCH], f32, tag="raw")
                        nc.scalar.copy(raw[:], pqk[:])
                        swp = tpool.tile([P, TCH], f32, tag="swp")
                        nc.sync.dma_start(swp[0:64, :], raw[64:128, :])
                        nc.sync.dma_start(swp[64:128, :], raw[0:64, :])
                        seg = qk_rt[:, fb, t0 : t0 + TCH]
                        tmpa = tpool.tile([P, TCH], f32, tag="tmpa")
                        nc.vector.tensor_mul(tmpa[:], pqk[:], cc_sb[:, t0 : t0 + TCH])
                        tmpb = tpool.tile([P, TCH], f32, tag="tmpb")
                        nc.vector.tensor_mul(tmpb[:], swp[:], ss_sb[:, t0 : t0 + TCH])
                        nc.vector.tensor_add(seg, tmpa[:], tmpb[:])
                        sq = spool.tile([P, TCH], bf16, tag="sq")
                        nc.vector.tensor_mul(sq[:], seg, seg)

                        def emit_stats(fb=fb, sq=sq):
                            pms = psms.tile([1, TCH], f32, tag="pms")
                            nc.tensor.matmul(
                                pms[:], ones_col[:], sq[:], start=True, stop=True
                            )
                            # rms = sqrt(ms/D + eps) on ACT (same table all phase)
                            rms = spool.tile([1, TCH], f32, tag="rms")
                            nc.scalar.activation(
                                rms[:], pms[:], AF.Sqrt, bias=eps_sb[0:1, :],
                                scale=1.0 / D,
                            )
                            return rms

                        def emit_finish(rms, fb=fb, seg=seg):
                            rstd = spool.tile([1, TCH], f32, tag="rstd")
                            nc.vector.reciprocal_approx_fast(rstd[:], rms[:])
                            rb = tpool.tile([P, TCH], f32, tag="rb")
                            nc.gpsimd.partition_broadcast(rb[:], rstd[:])
                            nc.vector.tensor_mul(seg, seg, rb[:])

                        stats_q.append(emit_stats)
                        finish_q.append(emit_finish)

                    # V blocks, natural layout: stationary = x block, moving = wv
                    def emit_v(tb, tch=tch, xt=xt):
                        pv = psv.tile([P, FK], f32, tag="pv")
                        for ko in range(KO):
                            nc.tensor.matmul(
                                pv[:], xt[:, ko, tb * P : (tb + 1) * P], wv_sb[:, ko, :],
                                start=(ko == 0), stop=(ko == KO - 1),
                            )
                        nc.vector.tensor_copy(v_sb[:, tch * (TCH // P) + tb, :], pv[:])

                    # PE emission order for this chunk: stats matmul for fb is
                    # emitted 2 fb-slots later, the rstd/apply chain 4 slots
                    # later, so the PE never waits on the DVE/ACT chains.
                    rms_tiles = [None] * NF
                    for fb in range(NF):
                        emit_fb(fb)
                        if 1 <= fb <= 4 and tch + 1 < NCHUNK:
                            dma_chunk_part(tch + 1, fb - 1)
                        if fb == 5 and tch + 1 < NCHUNK:
                            t1 = (tch + 1) * TCH
                            nc.sync.dma_start(cc_sb[:, t1 : t1 + TCH], cc[:, t1 : t1 + TCH])
                            nc.sync.dma_start(ss_sb[:, t1 : t1 + TCH], ss[:, t1 : t1 + TCH])
                        if fb >= 2:
                            rms_tiles[fb - 2] = stats_q[fb - 2]()
                        if fb >= 4:
                            finish_q[fb - 4](rms_tiles[fb - 4])
                    for tb in range(TCH // P):
                        emit_v(tb)
                        if tb < 2:
                            rms_tiles[NF - 2 + tb] = stats_q[NF - 2 + tb]()
                        finish_q[NF - 4 + tb](rms_tiles[NF - 4 + tb])

                nc.sync.dma_start(wo_sb[:], wo_r)

            # ---------------- Phase 2: attention + output projection ----------------
            pending_proj = []

            def make_proj_emitter(psy, ypool):
                def emit_proj_group():
                    tb, nch = pending_proj.pop(0)
                    yps = psy.tile([P, 512], f32, tag="yps", name="yps")
                    for h in range(NQ):
                        nc.tensor.matmul(
                            yps[:],
                            ot_sb[:, h, tb * P : (tb + 1) * P],
                            wo_sb[:, h, nch * 512 : (nch + 1) * 512],
                            start=(h == 0), stop=(h == NQ - 1),
                        )
                    ysb = ypool.tile([P, 512], bf16, tag="ysb", name="ysb")
                    nc.vector.tensor_copy(ysb[:], yps[:])
                    nc.sync.dma_start(
                        y[tb * P : (tb + 1) * P, nch * 512 : (nch + 1) * 512], ysb[:]
                    )
                return emit_proj_group

            with (
                tc.tile_pool(name="at_pt", bufs=4) as ptpool,
                tc.tile_pool(name="at_sb", bufs=3) as asb,
                tc.tile_pool(name="at_rb", bufs=2) as rbpool,
                tc.tile_pool(name="at_y", bufs=3) as ypool,
                tc.tile_pool(name="at_st", bufs=2, space="PSUM") as psst,
                tc.tile_pool(name="at_ot", bufs=2, space="PSUM") as psot,
                tc.tile_pool(name="at_sm", bufs=1, space="PSUM") as pssm,
                tc.tile_pool(name="at_yp", bufs=1, space="PSUM") as psy,
            ):
                emit_proj_group = make_proj_emitter(psy, ypool)

                for s in range(NSPAN):
                    q0 = s * SPAN
                    nkb = 4 * (s + 1)
                    npair = nkb // 2
                    span_iters = NQ * npair
                    span_pending = len(pending_proj)
                    it = 0

                    def off_of(kb, s=s):
                        return P * (kb - 4 * s) if kb >= 4 * s else 0

                    for h in range(NQ):
                        j = h // 2
                        q_ap = qk_rt[:, fb_of_q(h), q0 : q0 + SPAN]
                        ot_ps = psot.tile([P, SPAN], f32, tag="ot_ps")
                        sum_ps = pssm.tile([1, SPAN], f32, tag="sum_ps")

                        def emit_avsum(kb, pt_ap, h=h, j=j, nkb=nkb,
                                       ot_ps=ot_ps, sum_ps=sum_ps):
                            off = off_of(kb)
                            nc.tensor.matmul(
                                ot_ps[:, off:], v_sb[:, kb, j * D : (j + 1) * D],
                                pt_ap,
                                start=(kb == 0), stop=(kb == nkb - 1),
                                skip_group_check=True,
                            )
                            nc.tensor.matmul(
                                sum_ps[:, off:], ones_col[:], pt_ap,
                                start=(kb == 0), stop=(kb == nkb - 1),
                                skip_group_check=True,
                            )

                        prev = None  # (kb_a, pt_a_ap, kb_b, pt_b_ap)
                        for i in range(npair):
                            ka, kbb = 2 * i, 2 * i + 1
                            offa, offb = off_of(ka), off_of(kbb)
                            diag = kbb >= 4 * s
                            st2 = psst.tile([P, 2, SPAN], f32, tag="st2")
                            nc.tensor.matmul(
                                st2[:, 0, offa:],
                                qk_rt[:, j, ka * P : (ka + 1) * P],
                                q_ap[:, offa:],
                                start=True, stop=True,
                            )
                            nc.tensor.matmul(
                                st2[:, 1, offb:],
                                qk_rt[:, j, kbb * P : (kbb + 1) * P],
                                q_ap[:, offb:],
                                start=True, stop=True,
                            )
                            # PE filler while exp runs: previous pair's AV/sums
                            if prev is not None:
                                emit_avsum(prev[0], prev[1])
                                emit_avsum(prev[2], prev[3])
                            # spread the previous span's projection groups
                            # evenly over this span's iterations (PE filler
                            # that overlaps the exp chain)
                            n_pop = ((it + 1) * span_pending) // span_iters - (
                                it * span_pending
                            ) // span_iters
                            for _ in range(min(n_pop, len(pending_proj))):
                                emit_proj_group()
                            it += 1
                            # exp (+ mask on diagonal blocks)
                            pt2 = ptpool.tile([P, 2, SPAN], bf16, tag="pt2")
                            if diag:
                                nc.scalar.activation(
                                    pt2[:, 0, offa:], st2[:, 0, offa:], AF.Exp,
                                    scale=SCALE,
                                )
                                nc.scalar.activation(
                                    pt2[:, 1, offb:], st2[:, 1, offb:], AF.Exp,
                                    scale=SCALE,
                                )
                                if ka >= 4 * s:
                                    nc.vector.tensor_mul(
                                        pt2[:, 0, offa : offa + P],
                                        pt2[:, 0, offa : offa + P],
                                        mask_sb[:],
                                    )
                                nc.vector.tensor_mul(
                                    pt2[:, 1, offb : offb + P],
                                    pt2[:, 1, offb : offb + P],
                                    mask_sb[:],
                                )
                            else:
                                nc.scalar.activation(
                                    pt2[:, :, :], st2[:, :, :], AF.Exp, scale=SCALE
                                )
                            prev = (ka, pt2[:, 0, offa:], kbb, pt2[:, 1, offb:])
                        emit_avsum(prev[0], prev[1])
                        emit_avsum(prev[2], prev[3])

                        # normalization tail: 1/sums on DVE, bcast on GpSimd
                        rec = asb.tile([1, SPAN], f32, tag="rec")
                        nc.vector.reciprocal_approx_fast(rec[:], sum_ps[:])
                        rb = rbpool.tile([P, SPAN], f32, tag="rb2")
                        nc.gpsimd.partition_broadcast(rb[:], rec[:])
                        nc.vector.tensor_mul(
                            ot_sb[:, h, q0 : q0 + SPAN], ot_ps[:], rb[:]
                        )

                    # queue this span's projection; emitted inside span s+1's
                    # loop as PE filler (span 3's drains below)
                    for tb in range(4 * s, 4 * s + 4):
                        for nch in range(C // 512):
                            pending_proj.append((tb, nch))

            # span 3's projection: own PSUM pool (3 banks, freed by the
            # attention pools closing) so the groups pipeline through the
            # PSUM->SBUF copies without serializing
            with (
                tc.tile_pool(name="tl_y", bufs=3) as ypool2,
                tc.tile_pool(name="tl_yp", bufs=3, space="PSUM") as psy2,
            ):
                emit_tail_group = make_proj_emitter(psy2, ypool2)
                while pending_proj:
                    emit_tail_group()
    nc.compile()
    return nc


_NC_CACHE = None


def _get_nc():
    global _NC_CACHE
    if _NC_CACHE is None:
        _NC_CACHE = build()
    return _NC_CACHE


def _host_inputs(x, cos, sin, wq, wk, wv, wo):
    """Build the 8 per-core input maps."""
    bft = ml_dtypes.bfloat16
    cosT = np.ascontiguousarray(cos[0, :, 0, :].T).astype(np.float32)  # (64, T)
    sinT = np.ascontiguousarray(sin[0, :, 0, :].T).astype(np.float32)
    cc = np.concatenate([cosT, cosT], axis=0)          # (128, T)
    ss = np.concatenate([sinT, -sinT], axis=0)
    # mask[p, c] = 1 if c >= p (upper-tri incl diag): within a diagonal
    # 128-block, q-offset c sees key-offset p iff c >= p
    mask = (np.arange(P)[None, :] >= np.arange(P)[:, None]).astype(bft)

    xTs = [np.ascontiguousarray(x[b].T).astype(bft) for b in range(2)]
    wq16 = wq.astype(bft)
    wk16 = wk.astype(bft)
    wv16 = wv.astype(bft)
    wo16 = wo.astype(bft)
    in_maps = []
    for c in range(8):
        b, tp = divmod(c, 4)
        in_maps.append(
            {
                "xT": xTs[b],
                "wq": np.ascontiguousarray(wq16[:, tp * FQ : (tp + 1) * FQ]),
                "wk": np.ascontiguousarray(wk16[:, tp * FK : (tp + 1) * FK]),
                "wv": np.ascontiguousarray(wv16[:, tp * FK : (tp + 1) * FK]),
                "wo": np.ascontiguousarray(wo16[tp * FQ : (tp + 1) * FQ, :]),
                "cc": cc,
                "ss": ss,
                "mask": mask,
            }
        )
    return in_maps


def kernel(x, cos, sin, wq, wk, wv, wo, trace=False):
    x = np.asarray(x, dtype=np.float32)
    cos = np.asarray(cos, dtype=np.float32)
    sin = np.asarray(sin, dtype=np.float32)
    wq = np.asarray(wq, dtype=np.float32)
    wk = np.asarray(wk, dtype=np.float32)
    wv = np.asarray(wv, dtype=np.float32)
    wo = np.asarray(wo, dtype=np.float32)

    nc = _get_nc()
    in_maps = _host_inputs(x, cos, sin, wq, wk, wv, wo)
    res = run_bass_kernel_spmd(nc, in_maps, core_ids=list(range(8)), trace=trace)
    out = np.zeros((2, T, C), dtype=np.float32)
    for c in range(8):
        b = c // 4
        out[b] += res.results[c]["y"].astype(np.float32)
    if trace:
        return out, res
    return out


# revision 19
# speedup vs baseline: 1.3402x; 1.0072x over previous
"""Causal self-attention (RoPE + QK-RMSNorm, GQA 16q/8kv) Trainium2 Bass kernel.

Sharding: 8 cores = 2 batch x 4 tensor-parallel. Core c handles batch b=c//4 and
q-heads [4*tp, 4*tp+4), kv-heads [2*tp, 2*tp+2) where tp=c%4. Each core returns a
partial (T, C) output = O_heads @ wo[rows of its heads]; host sums the 4 partials
per batch (the "all-reduce after c_proj").

Perf notes vs the first working version:
- The scalar engine only ever runs Sqrt (phase 1) and Exp (attention) plus Copy,
  so there is no per-iteration activation-table reload.
- The PE stream is ordered so every cross-engine dependency has >1us of queued
  PE work in front of it: RMS stats matmuls are deferred one feature block,
  attention AV/sum matmuls run one kb-pair behind the score matmuls, and the
  output projection of span s-1 is interleaved into span s's loop as filler.
  This keeps the tensor engine continuously busy, which also keeps it at the
  2.4 GHz p-state (it drops to 1.2 GHz for 3us after every idle gap).
- V is produced directly in [t, feat] layout (x-block as stationary operand),
  removing the PE transposes.
- Softmax normalization: row sums via ones-column matmuls (PSUM), reciprocal on
  DVE (reciprocal_approx_fast), broadcast across partitions on GpSimd.
- Causal diagonal blocks are trimmed: score/exp/AV/sum only touch q-columns
  that can be live for that key block.
"""
import sys
import math

sys.path.insert(0, "/opt/trn_rl_repo")

import numpy as np
import ml_dtypes
import concourse.bacc as bacc
import concourse.mybir as mybir
import concourse.tile as tile
from concourse.bass_utils import run_bass_kernel_spmd

P = 128
T = 2048
C = 2048
KO = C // P          # 16 contraction tiles
D = 128              # head dim
NQ = 4               # q heads per core
NK = 2               # kv heads per core
NF = NQ + NK         # 6 rope/rms feature blocks (2 k + 4 q)
FQ = NQ * D          # 512
FK = NK * D          # 256
TCH = 512            # phase-1 T-chunk
NCHUNK = T // TCH    # 4
SPAN = 512           # attention q-span
NSPAN = T // SPAN    # 4
KB = T // P          # 16 key blocks
SCALE = 1.0 / math.sqrt(D)
EPS = 1.1920929e-07

f32 = mybir.dt.float32
bf16 = mybir.dt.bfloat16

AF = mybir.ActivationFunctionType

# feature-block order in qk_rt: k0, k1, q0..q3 (K first so the last chunk's
# K is ready the moment attention starts)
FB_K = [0, 1]

def fb_of_q(h):
    return NK + h


def build():
    nc = bacc.Bacc("TRN2", target_bir_lowering=False)
    xT = nc.dram_tensor("xT", (C, T), bf16, kind="ExternalInput")
    wq = nc.dram_tensor("wq", (C, FQ), bf16, kind="ExternalInput")
    wk = nc.dram_tensor("wk", (C, FK), bf16, kind="ExternalInput")
    wv = nc.dram_tensor("wv", (C, FK), bf16, kind="ExternalInput")
    wo = nc.dram_tensor("wo", (FQ, C), bf16, kind="ExternalInput")
    cc = nc.dram_tensor("cc", (P, T), f32, kind="ExternalInput")    # [cos; cos]
    ss = nc.dram_tensor("ss", (P, T), f32, kind="ExternalInput")    # [sin; -sin]
    mask = nc.dram_tensor("mask", (P, P), bf16, kind="ExternalInput")  # tri: c>=p
    y = nc.dram_tensor("y", (T, C), bf16, kind="ExternalOutput")

    xT_r = xT.rearrange("(ko p) t -> p ko t", p=P)
    wq_r = wq.rearrange("(ko p) f -> p ko f", p=P)
    wk_r = wk.rearrange("(ko p) f -> p ko f", p=P)
    wv_r = wv.rearrange("(ko p) f -> p ko f", p=P)
    wo_r = wo.rearrange("(ko p) n -> p ko n", p=P)

    with tile.TileContext(nc) as tc:
        with tc.tile_pool(name="persist", bufs=1) as persist:
            qk_rt = persist.tile([P, NF, T], bf16, tag="qk_rt")   # roped+normed kT/qT
            v_sb = persist.tile([P, KB, FK], bf16, tag="v_sb")    # V natural [t-part, kb, feat]
            cc_sb = persist.tile([P, T], f32, tag="cc_sb")
            ss_sb = persist.tile([P, T], f32, tag="ss_sb")
            mask_sb = persist.tile([P, P], bf16, tag="mask_sb")
            ones_col = persist.tile([P, 1], bf16, tag="ones_col")
            ones_f32 = persist.tile([P, 1], f32, tag="ones_f32")
            ot_sb = persist.tile([P, NQ, T], bf16, tag="ot_sb")
            wo_sb = persist.tile([P, NQ, C], bf16, tag="wo_sb")

            eps_sb = persist.tile([1, 1], f32, tag="eps_sb")
            nc.vector.memset(eps_sb[:], EPS)
            nc.vector.memset(ones_f32[:], 1.0)
            nc.vector.tensor_copy(ones_col[:], ones_f32[:])
            nc.sync.dma_start(mask_sb[:], mask[:, :])

            # ---------------- Phase 1: QKV + RoPE + RMS norm ----------------
            with (
                tc.tile_pool(name="ph1w", bufs=1) as wpool,
                tc.tile_pool(name="ph1x", bufs=2) as xpool,
                tc.tile_pool(name="ph1t", bufs=3) as tpool,
                tc.tile_pool(name="ph1s", bufs=3) as spool,
                tc.tile_pool(name="ph1ps", bufs=3, space="PSUM") as psqk,
                tc.tile_pool(name="ph1pv", bufs=2, space="PSUM") as psv,
                tc.tile_pool(name="ph1ms", bufs=2, space="PSUM") as psms,
            ):
                wq_sb = wpool.tile([P, KO, FQ], bf16, tag="wq_sb")
                wk_sb = wpool.tile([P, KO, FK], bf16, tag="wk_sb")
                wv_sb = wpool.tile([P, KO, FK], bf16, tag="wv_sb")

                # weight source AP per feature block (fb order: k0 k1 q0..q3)
                def w_ap(fb):
                    if fb < NK:
                        return wk_sb[:, :, fb * D : (fb + 1) * D]
                    h = fb - NK
                    return wq_sb[:, :, h * D : (h + 1) * D]

                xts = [None] * NCHUNK

                def dma_chunk_part(tch, part):
                    # split the 16 per-ko DMAs into 4 batches so latency-
                    # critical swap DMAs aren't queued behind them on sync
                    t0 = tch * TCH
                    if part == 0:
                        xts[tch] = xpool.tile([P, KO, TCH], bf16, tag="xt", name="xt")
                    xt = xts[tch]
                    for ko in range(4 * part, 4 * part + 4):
                        nc.sync.dma_start(xt[:, ko, :], xT_r[:, ko, t0 : t0 + TCH])

                # startup DMA priority order: the very first matmuls need only
                # wk slice j=0 and xt chunk-0, so emit those first and push the
                # q/v weights (needed 10+ us later) behind them
                nc.sync.dma_start(wk_sb[:, :, 0:D], wk_r[:, :, 0:D])
                dma_chunk_part(0, 0)
                dma_chunk_part(0, 1)
                nc.sync.dma_start(wk_sb[:, :, D : 2 * D], wk_r[:, :, D : 2 * D])
                dma_chunk_part(0, 2)
                dma_chunk_part(0, 3)
                nc.sync.dma_start(cc_sb[:, 0:TCH], cc[:, 0:TCH])
                nc.sync.dma_start(ss_sb[:, 0:TCH], ss[:, 0:TCH])
                for h in range(NQ):
                    nc.sync.dma_start(
                        wq_sb[:, :, h * D : (h + 1) * D], wq_r[:, :, h * D : (h + 1) * D]
                    )
                nc.sync.dma_start(wv_sb[:], wv_r)

                # deferred work queue: closures emitting the rope/stats/apply
                # chain pieces, run 1-2 fb slots after the matmuls they depend on
                for tch in range(NCHUNK):
                    t0 = tch * TCH
                    xt = xts[tch]
                    stats_q = []   # (emit_stats, emit_finish) per fb
                    finish_q = []

                    def emit_fb(fb, tch=tch, t0=t0, xt=xt):
                        pqk = psqk.tile([P, TCH], f32, tag="pqk")
                        for ko in range(KO):
                            nc.tensor.matmul(
                                pqk[:], w_ap(fb)[:, ko], xt[:, ko, :],
                                start=(ko == 0), stop=(ko == KO - 1),
                            )
                        # rope chain (ACT/DMA/DVE; runs while PE does next fb)
                        raw = tpool.tile([P, TCH], f32, tag="raw")
                        nc.scalar.copy(raw[:], pqk[:])
                        swp = tpool.tile([P, TCH], f32, tag="swp")
                        nc.sync.dma_start(swp[0:64, :], raw[64:128, :])
                        nc.sync.dma_start(swp[64:128, :], raw[0:64, :])
                        seg = qk_rt[:, fb, t0 : t0 + TCH]
                        tmpa = tpool.tile([P, TCH], f32, tag="tmpa")
                        nc.vector.tensor_mul(tmpa[:], pqk[:], cc_sb[:, t0 : t0 + TCH])
                        tmpb = tpool.tile([P, TCH], f32, tag="tmpb")
                        nc.vector.tensor_mul(tmpb[:], swp[:], ss_sb[:, t0 : t0 + TCH])
                        nc.vector.tensor_add(seg, tmpa[:], tmpb[:])
                        sq = spool.tile([P, TCH], bf16, tag="sq")
                        nc.vector.tensor_mul(sq[:], seg, seg)

                        def emit_stats(fb=fb, sq=sq):
                            pms = psms.tile([1, TCH], f32, tag="pms")
                            nc.tensor.matmul(
                                pms[:], ones_col[:], sq[:], start=True, stop=True
                            )
                            # rms = sqrt(ms/D + eps) on ACT (same table all phase)
                            rms = spool.tile([1, TCH], f32, tag="rms")
                            nc.scalar.activation(
                                rms[:], pms[:], AF.Sqrt, bias=eps_sb[0:1, :],
                                scale=1.0 / D,
                            )
                            return rms

                        def emit_finish(rms, fb=fb, seg=seg):
                            rstd = spool.tile([1, TCH], f32, tag="rstd")
                            nc.vector.reciprocal_approx_fast(rstd[:], rms[:])
                            rb = tpool.tile([P, TCH], f32, tag="rb")
                            nc.gpsimd.partition_broadcast(rb[:], rstd[:])
                            nc.vector.tensor_mul(seg, seg, rb[:])

                        stats_q.append(emit_stats)
                        finish_q.append(emit_finish)

                    # V blocks, natural layout: stationary = x block, moving = wv
                    def emit_v(tb, tch=tch, xt=xt):
                        pv = psv.tile([P, FK], f32, tag="pv")
                        for ko in range(KO):
                            nc.tensor.matmul(
                                pv[:], xt[:, ko, tb * P : (tb + 1) * P], wv_sb[:, ko, :],
                                start=(ko == 0), stop=(ko == KO - 1),
                            )
                        nc.vector.tensor_copy(v_sb[:, tch * (TCH // P) + tb, :], pv[:])

                    # PE emission order for this chunk: stats matmul for fb is
                    # emitted 2 fb-slots later, the rstd/apply chain 4 slots
                    # later, so the PE never waits on the DVE/ACT chains.
                    rms_tiles = [None] * NF
                    for fb in range(NF):
                        emit_fb(fb)
                        if 1 <= fb <= 4 and tch + 1 < NCHUNK:
                            dma_chunk_part(tch + 1, fb - 1)
                        if fb == 5 and tch + 1 < NCHUNK:
                            t1 = (tch + 1) * TCH
                            nc.sync.dma_start(cc_sb[:, t1 : t1 + TCH], cc[:, t1 : t1 + TCH])
                            nc.sync.dma_start(ss_sb[:, t1 : t1 + TCH], ss[:, t1 : t1 + TCH])
                        if fb >= 2:
                            rms_tiles[fb - 2] = stats_q[fb - 2]()
                        if fb >= 4:
                            finish_q[fb - 4](rms_tiles[fb - 4])
                    for tb in range(TCH // P):
                        emit_v(tb)
                        if tb < 2:
                            rms_tiles[NF - 2 + tb] = stats_q[NF - 2 + tb]()
                        finish_q[NF - 4 + tb](rms_tiles[NF - 4 + tb])

                nc.sync.dma_start(wo_sb[:], wo_r)

            # ---------------- Phase 2: attention + output projection ----------------
            pending_proj = []

            def make_proj_emitter(psy, ypool):
                def emit_proj_group():
                    tb, nch = pending_proj.pop(0)
                    yps = psy.tile([P, 512], f32, tag="yps", name="yps")
                    for h in range(NQ):
                        nc.tensor.matmul(
                            yps[:],
                            ot_sb[:, h, tb * P : (tb + 1) * P],
                            wo_sb[:, h, nch * 512 : (nch + 1) * 512],
                            start=(h == 0), stop=(h == NQ - 1),
                        )
                    ysb = ypool.tile([P, 512], bf16, tag="ysb", name="ysb")
                    nc.vector.tensor_copy(ysb[:], yps[:])
                    nc.sync.dma_start(
                        y[tb * P : (tb + 1) * P, nch * 512 : (nch + 1) * 512], ysb[:]
                    )
                return emit_proj_group

            with (
                tc.tile_pool(name="at_pt", bufs=4) as ptpool,
                tc.tile_pool(name="at_sb", bufs=3) as asb,
                tc.tile_pool(name="at_rb", bufs=2) as rbpool,
                tc.tile_pool(name="at_y", bufs=3) as ypool,
                tc.tile_pool(name="at_st", bufs=2, space="PSUM") as psst,
                tc.tile_pool(name="at_ot", bufs=2, space="PSUM") as psot,
                tc.tile_pool(name="at_sm", bufs=1, space="PSUM") as pssm,
                tc.tile_pool(name="at_yp", bufs=1, space="PSUM") as psy,
            ):
                emit_proj_group = make_proj_emitter(psy, ypool)

                for s in range(NSPAN):
                    q0 = s * SPAN
                    nkb = 4 * (s + 1)
                    npair = nkb // 2
                    span_iters = NQ * npair
                    span_pending = len(pending_proj)
                    it = 0

                    def off_of(kb, s=s):
                        return P * (kb - 4 * s) if kb >= 4 * s else 0

                    for h in range(NQ):
                        j = h // 2
                        q_ap = qk_rt[:, fb_of_q(h), q0 : q0 + SPAN]
                        ot_ps = psot.tile([P, SPAN], f32, tag="ot_ps")
                        sum_ps = pssm.tile([1, SPAN], f32, tag="sum_ps")

                        def emit_avsum(kb, pt_ap, h=h, j=j, nkb=nkb,
                                       ot_ps=ot_ps, sum_ps=sum_ps):
                            off = off_of(kb)
                            nc.tensor.matmul(
                                ot_ps[:, off:], v_sb[:, kb, j * D : (j + 1) * D],
                                pt_ap,
                                start=(kb == 0), stop=(kb == nkb - 1),
                                skip_group_check=True,
                            )
                            nc.tensor.matmul(
                                sum_ps[:, off:], ones_col[:], pt_ap,
                                start=(kb == 0), stop=(kb == nkb - 1),
                                skip_group_check=True,
                            )

                        # AV/sum run TWO kb-pairs behind the scores so the
                        # exp chain (plus its two semaphore hops) never gates
                        # the PE
                        pipe = []  # queued (kb_a, pt_a_ap, kb_b, pt_b_ap)
                        for i in range(npair):
                            ka, kbb = 2 * i, 2 * i + 1
                            offa, offb = off_of(ka), off_of(kbb)
                            diag = kbb >= 4 * s
                            st2 = psst.tile([P, 2, SPAN], f32, tag="st2")
                            nc.tensor.matmul(
                                st2[:, 0, offa:],
                                qk_rt[:, j, ka * P : (ka + 1) * P],
                                q_ap[:, offa:],
                                start=True, stop=True,
                            )
                            nc.tensor.matmul(
                                st2[:, 1, offb:],
                                qk_rt[:, j, kbb * P : (kbb + 1) * P],
                                q_ap[:, offb:],
                                start=True, stop=True,
                            )
                            # PE filler while exp runs: AV/sums from 2 pairs ago
                            if len(pipe) >= 2:
                                pa = pipe.pop(0)
                                emit_avsum(pa[0], pa[1])
                                emit_avsum(pa[2], pa[3])
                            # spread the previous span's projection groups
                            # evenly over this span's iterations (PE filler
                            # that overlaps the exp chain)
                            n_pop = ((it + 1) * span_pending) // span_iters - (
                                it * span_pending
                            ) // span_iters
                            for _ in range(min(n_pop, len(pending_proj))):
                                emit_proj_group()
                            it += 1
                            # exp (+ mask on diagonal blocks)
                            pt2 = ptpool.tile([P, 2, SPAN], bf16, tag="pt2")
                            if diag:
                                nc.scalar.activation(
                                    pt2[:, 0, offa:], st2[:, 0, offa:], AF.Exp,
                                    scale=SCALE,
                                )
                                nc.scalar.activation(
                                    pt2[:, 1, offb:], st2[:, 1, offb:], AF.Exp,
                                    scale=SCALE,
                                )
                                if ka >= 4 * s:
                                    nc.vector.tensor_mul(
                                        pt2[:, 0, offa : offa + P],
                                        pt2[:, 0, offa : offa + P],
                                        mask_sb[:],
                                    )
                                nc.vector.tensor_mul(
                                    pt2[:, 1, offb : offb + P],
                                    pt2[:, 1, offb : offb + P],
                                    mask_sb[:],
                                )
                            else:
                                nc.scalar.activation(
                                    pt2[:, :, :], st2[:, :, :], AF.Exp, scale=SCALE
                                )
                            pipe.append((ka, pt2[:, 0, offa:], kbb, pt2[:, 1, offb:]))
                        for pa in pipe:
                            emit_avsum(pa[0], pa[1])
                            emit_avsum(pa[2], pa[3])

                        # normalization tail: 1/sums on DVE, bcast on GpSimd
                        rec = asb.tile([1, SPAN], f32, tag="rec")
                        nc.vector.reciprocal_approx_fast(rec[:], sum_ps[:])
                        rb = rbpool.tile([P, SPAN], f32, tag="rb2")
                        nc.gpsimd.partition_broadcast(rb[:], rec[:])
                        nc.vector.tensor_mul(
                            ot_sb[:, h, q0 : q0 + SPAN], ot_ps[:], rb[:]
                        )

                    # queue this span's projection; emitted inside span s+1's
                    # loop as PE filler (span 3's drains below)
                    for tb in range(4 * s, 4 * s + 4):
                        for nch in range(C // 512):
                            pending_proj.append((tb, nch))

            # span 3's projection: own PSUM pool (3 banks, freed by the
            # attention pools closing) so the groups pipeline through the
            # PSUM->SBUF copies without serializing
            with (
                tc.tile_pool(name="tl_y", bufs=3) as ypool2,
                tc.tile_pool(name="tl_yp", bufs=3, space="PSUM") as psy2,
            ):
                emit_tail_group = make_proj_emitter(psy2, ypool2)
                while pending_proj:
                    emit_tail_group()
    nc.compile()
    return nc


_NC_CACHE = None


def _get_nc():
    global _NC_CACHE
    if _NC_CACHE is None:
        _NC_CACHE = build()
    return _NC_CACHE


def _host_inputs(x, cos, sin, wq, wk, wv, wo):
    """Build the 8 per-core input maps."""
    bft = ml_dtypes.bfloat16
    cosT = np.ascontiguousarray(cos[0, :, 0, :].T).astype(np.float32)  # (64, T)
    sinT = np.ascontiguousarray(sin[0, :, 0, :].T).astype(np.float32)
    cc = np.concatenate([cosT, cosT], axis=0)          # (128, T)
    ss = np.concatenate([sinT, -sinT], axis=0)
    # mask[p, c] = 1 if c >= p (upper-tri incl diag): within a diagonal
    # 128-block, q-offset c sees key-offset p iff c >= p
    mask = (np.arange(P)[None, :] >= np.arange(P)[:, None]).astype(bft)

    xTs = [np.ascontiguousarray(x[b].T).astype(bft) for b in range(2)]
    wq16 = wq.astype(bft)
    wk16 = wk.astype(bft)
    wv16 = wv.astype(bft)
    wo16 = wo.astype(bft)
    in_maps = []
    for c in range(8):
        b, tp = divmod(c, 4)
        in_maps.append(
            {
                "xT": xTs[b],
                "wq": np.ascontiguousarray(wq16[:, tp * FQ : (tp + 1) * FQ]),
                "wk": np.ascontiguousarray(wk16[:, tp * FK : (tp + 1) * FK]),
                "wv": np.ascontiguousarray(wv16[:, tp * FK : (tp + 1) * FK]),
                "wo": np.ascontiguousarray(wo16[tp * FQ : (tp + 1) * FQ, :]),
                "cc": cc,
                "ss": ss,
                "mask": mask,
            }
        )
    return in_maps


def kernel(x, cos, sin, wq, wk, wv, wo, trace=False):
    x = np.asarray(x, dtype=np.float32)
    cos = np.asarray(cos, dtype=np.float32)
    sin = np.asarray(sin, dtype=np.float32)
    wq = np.asarray(wq, dtype=np.float32)
    wk = np.asarray(wk, dtype=np.float32)
    wv = np.asarray(wv, dtype=np.float32)
    wo = np.asarray(wo, dtype=np.float32)

    nc = _get_nc()
    in_maps = _host_inputs(x, cos, sin, wq, wk, wv, wo)
    res = run_bass_kernel_spmd(nc, in_maps, core_ids=list(range(8)), trace=trace)
    out = np.zeros((2, T, C), dtype=np.float32)
    for c in range(8):
        b = c // 4
        out[b] += res.results[c]["y"].astype(np.float32)
    if trace:
        return out, res
    return out


# revision 22
# speedup vs baseline: 1.4077x; 1.0503x over previous
"""Causal self-attention (RoPE + QK-RMSNorm, GQA 16q/8kv) Trainium2 Bass kernel.

Sharding: 8 cores = 2 batch x 4 tensor-parallel. Core c handles batch b=c//4 and
q-heads [4*tp, 4*tp+4), kv-heads [2*tp, 2*tp+2) where tp=c%4. Each core returns a
partial (T, C) output = O_heads @ wo[rows of its heads]; host sums the 4 partials
per batch (the "all-reduce after c_proj").

Perf notes vs the first working version:
- The scalar engine only ever runs Sqrt (phase 1) and Exp (attention) plus Copy,
  so there is no per-iteration activation-table reload.
- The PE stream is ordered so every cross-engine dependency has >1us of queued
  PE work in front of it: RMS stats matmuls are deferred one feature block,
  attention AV/sum matmuls run one kb-pair behind the score matmuls, and the
  output projection of span s-1 is interleaved into span s's loop as filler.
  This keeps the tensor engine continuously busy, which also keeps it at the
  2.4 GHz p-state (it drops to 1.2 GHz for 3us after every idle gap).
- V is produced directly in [t, feat] layout (x-block as stationary operand),
  removing the PE transposes.
- Softmax normalization: row sums via ones-column matmuls (PSUM), reciprocal on
  DVE (reciprocal_approx_fast), broadcast across partitions on GpSimd.
- Causal diagonal blocks are trimmed: score/exp/AV/sum only touch q-columns
  that can be live for that key block.
"""
import sys
import math

sys.path.insert(0, "/opt/trn_rl_repo")

import numpy as np
import ml_dtypes
import concourse.bacc as bacc
import concourse.mybir as mybir
import concourse.tile as tile
from concourse.bass_utils import run_bass_kernel_spmd

P = 128
T = 2048
C = 2048
KO = C // P          # 16 contraction tiles
D = 128              # head dim
NQ = 4               # q heads per core
NK = 2               # kv heads per core
NF = NQ + NK         # 6 rope/rms feature blocks (2 k + 4 q)
FQ = NQ * D          # 512
FK = NK * D          # 256
TCH = 512            # phase-1 T-chunk
NCHUNK = T // TCH    # 4
SPAN = 512           # attention q-span
NSPAN = T // SPAN    # 4
KB = T // P          # 16 key blocks
SCALE = 1.0 / math.sqrt(D)
EPS = 1.1920929e-07

f32 = mybir.dt.float32
bf16 = mybir.dt.bfloat16

AF = mybir.ActivationFunctionType

# feature-block order in qk_rt: k0, k1, q0..q3 (K first so the last chunk's
# K is ready the moment attention starts)
FB_K = [0, 1]

def fb_of_q(h):
    return NK + h


def build():
    nc = bacc.Bacc("TRN2", target_bir_lowering=False)
    xT = nc.dram_tensor("xT", (C, T), bf16, kind="ExternalInput")
    wq = nc.dram_tensor("wq", (C, FQ), bf16, kind="ExternalInput")
    wk = nc.dram_tensor("wk", (C, FK), bf16, kind="ExternalInput")
    wv = nc.dram_tensor("wv", (C, FK), bf16, kind="ExternalInput")
    wo = nc.dram_tensor("wo", (FQ, C), bf16, kind="ExternalInput")
    cc = nc.dram_tensor("cc", (P, T), f32, kind="ExternalInput")    # [cos; cos]
    ss = nc.dram_tensor("ss", (P, T), f32, kind="ExternalInput")    # [sin; -sin]
    mask = nc.dram_tensor("mask", (P, P), bf16, kind="ExternalInput")  # tri: c>=p
    y = nc.dram_tensor("y", (T, C), bf16, kind="ExternalOutput")

    xT_r = xT.rearrange("(ko p) t -> p ko t", p=P)
    wq_r = wq.rearrange("(ko p) f -> p ko f", p=P)
    wk_r = wk.rearrange("(ko p) f -> p ko f", p=P)
    wv_r = wv.rearrange("(ko p) f -> p ko f", p=P)
    wo_r = wo.rearrange("(ko p) n -> p ko n", p=P)

    with tile.TileContext(nc) as tc:
        with tc.tile_pool(name="persist", bufs=1) as persist:
            qk_rt = persist.tile([P, NF, T], bf16, tag="qk_rt")   # roped+normed kT/qT
            v_sb = persist.tile([P, KB, FK], bf16, tag="v_sb")    # V natural [t-part, kb, feat]
            cc_sb = persist.tile([P, T], f32, tag="cc_sb")
            ss_sb = persist.tile([P, T], f32, tag="ss_sb")
            mask_sb = persist.tile([P, P], bf16, tag="mask_sb")
            ones_col = persist.tile([P, 1], bf16, tag="ones_col")
            ones_f32 = persist.tile([P, 1], f32, tag="ones_f32")
            ot_sb = persist.tile([P, NQ, T], bf16, tag="ot_sb")
            wo_sb = persist.tile([P, NQ, C], bf16, tag="wo_sb")

            eps_sb = persist.tile([1, 1], f32, tag="eps_sb")
            nc.vector.memset(eps_sb[:], EPS)
            nc.vector.memset(ones_f32[:], 1.0)
            nc.vector.tensor_copy(ones_col[:], ones_f32[:])
            nc.sync.dma_start(mask_sb[:], mask[:, :])

            # ---------------- Phase 1: QKV + RoPE + RMS norm ----------------
            with (
                tc.tile_pool(name="ph1w", bufs=1) as wpool,
                tc.tile_pool(name="ph1x", bufs=2) as xpool,
                tc.tile_pool(name="ph1t", bufs=3) as tpool,
                tc.tile_pool(name="ph1s", bufs=3) as spool,
                tc.tile_pool(name="ph1ps", bufs=3, space="PSUM") as psqk,
                tc.tile_pool(name="ph1pv", bufs=2, space="PSUM") as psv,
                tc.tile_pool(name="ph1ms", bufs=2, space="PSUM") as psms,
            ):
                wq_sb = wpool.tile([P, KO, FQ], bf16, tag="wq_sb")
                wk_sb = wpool.tile([P, KO, FK], bf16, tag="wk_sb")
                wv_sb = wpool.tile([P, KO, FK], bf16, tag="wv_sb")

                # weight source AP per feature block (fb order: k0 k1 q0..q3)
                def w_ap(fb):
                    if fb < NK:
                        return wk_sb[:, :, fb * D : (fb + 1) * D]
                    h = fb - NK
                    return wq_sb[:, :, h * D : (h + 1) * D]

                xts = [None] * NCHUNK

                def dma_chunk_part(tch, part):
                    # split the 16 per-ko DMAs into 4 batches so latency-
                    # critical swap DMAs aren't queued behind them on sync
                    t0 = tch * TCH
                    if part == 0:
                        xts[tch] = xpool.tile([P, KO, TCH], bf16, tag="xt", name="xt")
                    xt = xts[tch]
                    for ko in range(4 * part, 4 * part + 4):
                        nc.sync.dma_start(xt[:, ko, :], xT_r[:, ko, t0 : t0 + TCH])

                # startup DMA priority order: the very first matmuls need only
                # wk slice j=0 and xt chunk-0, so emit those first and push the
                # q/v weights (needed 10+ us later) behind them
                nc.sync.dma_start(wk_sb[:, :, 0:D], wk_r[:, :, 0:D])
                dma_chunk_part(0, 0)
                dma_chunk_part(0, 1)
                nc.sync.dma_start(wk_sb[:, :, D : 2 * D], wk_r[:, :, D : 2 * D])
                dma_chunk_part(0, 2)
                dma_chunk_part(0, 3)
                nc.sync.dma_start(cc_sb[:, 0:TCH], cc[:, 0:TCH])
                nc.sync.dma_start(ss_sb[:, 0:TCH], ss[:, 0:TCH])
                for h in range(NQ):
                    nc.sync.dma_start(
                        wq_sb[:, :, h * D : (h + 1) * D], wq_r[:, :, h * D : (h + 1) * D]
                    )
                nc.sync.dma_start(wv_sb[:], wv_r)

                # deferred work queue: closures emitting the rope/stats/apply
                # chain pieces, run 1-2 fb slots after the matmuls they depend on
                for tch in range(NCHUNK):
                    t0 = tch * TCH
                    xt = xts[tch]
                    stats_q = []   # (emit_stats, emit_finish) per fb
                    finish_q = []

                    def emit_fb(fb, tch=tch, t0=t0, xt=xt):
                        pqk = psqk.tile([P, TCH], f32, tag="pqk")
                        for ko in range(KO):
                            nc.tensor.matmul(
                                pqk[:], w_ap(fb)[:, ko], xt[:, ko, :],
                                start=(ko == 0), stop=(ko == KO - 1),
                            )
                        # rope chain (ACT/DMA/DVE; runs while PE does next fb)
                        raw = tpool.tile([P, TCH], f32, tag="raw")
                        nc.scalar.copy(raw[:], pqk[:])
                        swp = tpool.tile([P, TCH], f32, tag="swp")
                        nc.sync.dma_start(swp[0:64, :], raw[64:128, :])
                        nc.sync.dma_start(swp[64:128, :], raw[0:64, :])
                        seg = qk_rt[:, fb, t0 : t0 + TCH]
                        tmpa = tpool.tile([P, TCH], f32, tag="tmpa")
                        nc.vector.tensor_mul(tmpa[:], pqk[:], cc_sb[:, t0 : t0 + TCH])
                        tmpb = tpool.tile([P, TCH], f32, tag="tmpb")
                        nc.vector.tensor_mul(tmpb[:], swp[:], ss_sb[:, t0 : t0 + TCH])
                        nc.vector.tensor_add(seg, tmpa[:], tmpb[:])
                        sq = spool.tile([P, TCH], bf16, tag="sq")
                        nc.vector.tensor_mul(sq[:], seg, seg)

                        def emit_stats(fb=fb, sq=sq):
                            pms = psms.tile([1, TCH], f32, tag="pms")
                            nc.tensor.matmul(
                                pms[:], ones_col[:], sq[:], start=True, stop=True
                            )
                            # rms = sqrt(ms/D + eps) on ACT (same table all phase)
                            rms = spool.tile([1, TCH], f32, tag="rms")
                            nc.scalar.activation(
                                rms[:], pms[:], AF.Sqrt, bias=eps_sb[0:1, :],
                                scale=1.0 / D,
                            )
                            return rms

                        def emit_finish(rms, fb=fb, seg=seg):
                            rstd = spool.tile([1, TCH], f32, tag="rstd")
                            nc.vector.reciprocal_approx_fast(rstd[:], rms[:])
                            rb = tpool.tile([P, TCH], f32, tag="rb")
                            nc.gpsimd.partition_broadcast(rb[:], rstd[:])
                            nc.vector.tensor_mul(seg, seg, rb[:])

                        stats_q.append(emit_stats)
                        finish_q.append(emit_finish)

                    # V blocks, natural layout: stationary = x block, moving = wv
                    def emit_v(tb, tch=tch, xt=xt):
                        pv = psv.tile([P, FK], f32, tag="pv")
                        for ko in range(KO):
                            nc.tensor.matmul(
                                pv[:], xt[:, ko, tb * P : (tb + 1) * P], wv_sb[:, ko, :],
                                start=(ko == 0), stop=(ko == KO - 1),
                            )
                        nc.vector.tensor_copy(v_sb[:, tch * (TCH // P) + tb, :], pv[:])

                    # PE emission order for this chunk: stats matmul for fb is
                    # emitted 2 fb-slots later, the rstd/apply chain 4 slots
                    # later, so the PE never waits on the DVE/ACT chains.
                    rms_tiles = [None] * NF
                    for fb in range(NF):
                        emit_fb(fb)
                        if 1 <= fb <= 4 and tch + 1 < NCHUNK:
                            dma_chunk_part(tch + 1, fb - 1)
                        if fb == 5 and tch + 1 < NCHUNK:
                            t1 = (tch + 1) * TCH
                            nc.sync.dma_start(cc_sb[:, t1 : t1 + TCH], cc[:, t1 : t1 + TCH])
                            nc.sync.dma_start(ss_sb[:, t1 : t1 + TCH], ss[:, t1 : t1 + TCH])
                        if fb >= 2:
                            rms_tiles[fb - 2] = stats_q[fb - 2]()
                        if fb >= 4:
                            finish_q[fb - 4](rms_tiles[fb - 4])
                    for tb in range(TCH // P):
                        emit_v(tb)
                        if tb < 2:
                            rms_tiles[NF - 2 + tb] = stats_q[NF - 2 + tb]()
                        finish_q[NF - 4 + tb](rms_tiles[NF - 4 + tb])

                nc.sync.dma_start(wo_sb[:], wo_r)

            # ---------------- Phase 2: attention + output projection ----------------
            pending_proj = []

            def make_proj_emitter(psy, ypool):
                def emit_proj_group():
                    tb, nch = pending_proj.pop(0)
                    yps = psy.tile([P, 512], f32, tag="yps", name="yps")
                    for h in range(NQ):
                        nc.tensor.matmul(
                            yps[:],
                            ot_sb[:, h, tb * P : (tb + 1) * P],
                            wo_sb[:, h, nch * 512 : (nch + 1) * 512],
                            start=(h == 0), stop=(h == NQ - 1),
                        )
                    ysb = ypool.tile([P, 512], bf16, tag="ysb", name="ysb")
                    nc.vector.tensor_copy(ysb[:], yps[:])
                    nc.sync.dma_start(
                        y[tb * P : (tb + 1) * P, nch * 512 : (nch + 1) * 512], ysb[:]
                    )
                return emit_proj_group

            with (
                tc.tile_pool(name="at_pt", bufs=4) as ptpool,
                tc.tile_pool(name="at_sb", bufs=3) as asb,
                tc.tile_pool(name="at_rb", bufs=2) as rbpool,
                tc.tile_pool(name="at_y", bufs=3) as ypool,
                tc.tile_pool(name="at_st", bufs=2, space="PSUM") as psst,
                tc.tile_pool(name="at_ot", bufs=2, space="PSUM") as psot,
                tc.tile_pool(name="at_sm", bufs=1, space="PSUM") as pssm,
                tc.tile_pool(name="at_yp", bufs=1, space="PSUM") as psy,
            ):
                emit_proj_group = make_proj_emitter(psy, ypool)

                for s in range(NSPAN):
                    q0 = s * SPAN
                    nkb = 4 * (s + 1)
                    npair = nkb // 2
                    span_iters = NQ * npair
                    span_pending = len(pending_proj)
                    it = 0

                    def off_of(kb, s=s):
                        return P * (kb - 4 * s) if kb >= 4 * s else 0

                    for h in range(NQ):
                        j = h // 2
                        q_ap = qk_rt[:, fb_of_q(h), q0 : q0 + SPAN]
                        ot_ps = psot.tile([P, SPAN], f32, tag="ot_ps")
                        sum_ps = pssm.tile([1, SPAN], f32, tag="sum_ps")

                        def emit_avsum(ka, pt_a, kbb, pt_b, h=h, j=j, nkb=nkb,
                                       ot_ps=ot_ps, sum_ps=sum_ps):
                            # group by destination bank: accumulate-mode bank
                            # switches cost ~80ns of PE turnaround each
                            offa, offb = off_of(ka), off_of(kbb)
                            nc.tensor.matmul(
                                ot_ps[:, offa:], v_sb[:, ka, j * D : (j + 1) * D],
                                pt_a,
                                start=(ka == 0), stop=False,
                                skip_group_check=True,
                            )
                            nc.tensor.matmul(
                                ot_ps[:, offb:], v_sb[:, kbb, j * D : (j + 1) * D],
                                pt_b,
                                start=False, stop=(kbb == nkb - 1),
                                skip_group_check=True,
                            )
                            nc.tensor.matmul(
                                sum_ps[:, offa:], ones_col[:], pt_a,
                                start=(ka == 0), stop=False,
                                skip_group_check=True,
                            )
                            nc.tensor.matmul(
                                sum_ps[:, offb:], ones_col[:], pt_b,
                                start=False, stop=(kbb == nkb - 1),
                                skip_group_check=True,
                            )

                        # AV/sum run TWO kb-pairs behind the scores so the
                        # exp chain (plus its two semaphore hops) never gates
                        # the PE
                        pipe = []  # queued (kb_a, pt_a_ap, kb_b, pt_b_ap)
                        for i in range(npair):
                            ka, kbb = 2 * i, 2 * i + 1
                            offa, offb = off_of(ka), off_of(kbb)
                            diag = kbb >= 4 * s
                            st2 = psst.tile([P, 2, SPAN], f32, tag="st2")
                            nc.tensor.matmul(
                                st2[:, 0, offa:],
                                qk_rt[:, j, ka * P : (ka + 1) * P],
                                q_ap[:, offa:],
                                start=True, stop=True,
                            )
                            nc.tensor.matmul(
                                st2[:, 1, offb:],
                                qk_rt[:, j, kbb * P : (kbb + 1) * P],
                                q_ap[:, offb:],
                                start=True, stop=True,
                            )
                            # PE filler while exp runs: AV/sums from 2 pairs ago
                            if len(pipe) >= 2:
                                pa = pipe.pop(0)
                                emit_avsum(*pa)
                            # spread the previous span's projection groups
                            # evenly over this span's iterations (PE filler
                            # that overlaps the exp chain)
                            n_pop = ((it + 1) * span_pending) // span_iters - (
                                it * span_pending
                            ) // span_iters
                            for _ in range(min(n_pop, len(pending_proj))):
                                emit_proj_group()
                            it += 1
                            # exp (+ mask on diagonal blocks)
                            pt2 = ptpool.tile([P, 2, SPAN], bf16, tag="pt2")
                            if diag:
                                nc.scalar.activation(
                                    pt2[:, 0, offa:], st2[:, 0, offa:], AF.Exp,
                                    scale=SCALE,
                                )
                                nc.scalar.activation(
                                    pt2[:, 1, offb:], st2[:, 1, offb:], AF.Exp,
                                    scale=SCALE,
                                )
                                if ka >= 4 * s:
                                    nc.vector.tensor_mul(
                                        pt2[:, 0, offa : offa + P],
                                        pt2[:, 0, offa : offa + P],
                                        mask_sb[:],
                                    )
                                nc.vector.tensor_mul(
                                    pt2[:, 1, offb : offb + P],
                                    pt2[:, 1, offb : offb + P],
                                    mask_sb[:],
                                )
                            else:
                                nc.scalar.activation(
                                    pt2[:, :, :], st2[:, :, :], AF.Exp, scale=SCALE
                                )
                            pipe.append((ka, pt2[:, 0, offa:], kbb, pt2[:, 1, offb:]))
                        for pa in pipe:
                            emit_avsum(*pa)

                        # normalization tail: 1/sums on DVE, bcast on GpSimd
                        rec = asb.tile([1, SPAN], f32, tag="rec")
                        nc.vector.reciprocal_approx_fast(rec[:], sum_ps[:])
                        rb = rbpool.tile([P, SPAN], f32, tag="rb2")
                        nc.gpsimd.partition_broadcast(rb[:], rec[:])
                        nc.vector.tensor_mul(
                            ot_sb[:, h, q0 : q0 + SPAN], ot_ps[:], rb[:]
                        )

                    # queue this span's projection; emitted inside span s+1's
                    # loop as PE filler (span 3's drains below)
                    for tb in range(4 * s, 4 * s + 4):
                        for nch in range(C // 512):
                            pending_proj.append((tb, nch))

            # span 3's projection: own PSUM pool (3 banks, freed by the
            # attention pools closing) so the groups pipeline through the
            # PSUM->SBUF copies without serializing
            with (
                tc.tile_pool(name="tl_y", bufs=3) as ypool2,
                tc.tile_pool(name="tl_yp", bufs=3, space="PSUM") as psy2,
            ):
                emit_tail_group = make_proj_emitter(psy2, ypool2)
                while pending_proj:
                    emit_tail_group()
    nc.compile()
    return nc


_NC_CACHE = None


def _get_nc():
    global _NC_CACHE
    if _NC_CACHE is None:
        _NC_CACHE = build()
    return _NC_CACHE


def _host_inputs(x, cos, sin, wq, wk, wv, wo):
    """Build the 8 per-core input maps."""
    bft = ml_dtypes.bfloat16
    cosT = np.ascontiguousarray(cos[0, :, 0, :].T).astype(np.float32)  # (64, T)
    sinT = np.ascontiguousarray(sin[0, :, 0, :].T).astype(np.float32)
    cc = np.concatenate([cosT, cosT], axis=0)          # (128, T)
    ss = np.concatenate([sinT, -sinT], axis=0)
    # mask[p, c] = 1 if c >= p (upper-tri incl diag): within a diagonal
    # 128-block, q-offset c sees key-offset p iff c >= p
    mask = (np.arange(P)[None, :] >= np.arange(P)[:, None]).astype(bft)

    xTs = [np.ascontiguousarray(x[b].T).astype(bft) for b in range(2)]
    wq16 = wq.astype(bft)
    wk16 = wk.astype(bft)
    wv16 = wv.astype(bft)
    wo16 = wo.astype(bft)
    in_maps = []
    for c in range(8):
        b, tp = divmod(c, 4)
        in_maps.append(
            {
                "xT": xTs[b],
                "wq": np.ascontiguousarray(wq16[:, tp * FQ : (tp + 1) * FQ]),
                "wk": np.ascontiguousarray(wk16[:, tp * FK : (tp + 1) * FK]),
                "wv": np.ascontiguousarray(wv16[:, tp * FK : (tp + 1) * FK]),
                "wo": np.ascontiguousarray(wo16[tp * FQ : (tp + 1) * FQ, :]),
                "cc": cc,
                "ss": ss,
                "mask": mask,
            }
        )
    return in_maps


def kernel(x, cos, sin, wq, wk, wv, wo, trace=False):
    x = np.asarray(x, dtype=np.float32)
    cos = np.asarray(cos, dtype=np.float32)
    sin = np.asarray(sin, dtype=np.float32)
    wq = np.asarray(wq, dtype=np.float32)
    wk = np.asarray(wk, dtype=np.float32)
    wv = np.asarray(wv, dtype=np.float32)
    wo = np.asarray(wo, dtype=np.float32)

    nc = _get_nc()
    in_maps = _host_inputs(x, cos, sin, wq, wk, wv, wo)
    res = run_bass_kernel_spmd(nc, in_maps, core_ids=list(range(8)), trace=trace)
    out = np.zeros((2, T, C), dtype=np.float32)
    for c in range(8):
        b = c // 4
        out[b] += res.results[c]["y"].astype(np.float32)
    if trace:
        return out, res
    return out


# revision 27
# speedup vs baseline: 1.4295x; 1.0155x over previous
"""Causal self-attention (RoPE + QK-RMSNorm, GQA 16q/8kv) Trainium2 Bass kernel.

Sharding: 8 cores = 2 batch x 4 tensor-parallel. Core c handles batch b=c//4 and
q-heads [4*tp, 4*tp+4), kv-heads [2*tp, 2*tp+2) where tp=c%4. Each core returns a
partial (T, C) output = O_heads @ wo[rows of its heads]; host sums the 4 partials
per batch (the "all-reduce after c_proj").

Perf notes vs the first working version:
- The scalar engine only ever runs Sqrt (phase 1) and Exp (attention) plus Copy,
  so there is no per-iteration activation-table reload.
- The PE stream is ordered so every cross-engine dependency has >1us of queued
  PE work in front of it: RMS stats matmuls are deferred one feature block,
  attention AV/sum matmuls run one kb-pair behind the score matmuls, and the
  output projection of span s-1 is interleaved into span s's loop as filler.
  This keeps the tensor engine continuously busy, which also keeps it at the
  2.4 GHz p-state (it drops to 1.2 GHz for 3us after every idle gap).
- V is produced directly in [t, feat] layout (x-block as stationary operand),
  removing the PE transposes.
- Softmax normalization: row sums via ones-column matmuls (PSUM), reciprocal on
  DVE (reciprocal_approx_fast), broadcast across partitions on GpSimd.
- Causal diagonal blocks are trimmed: score/exp/AV/sum only touch q-columns
  that can be live for that key block.
"""
import sys
import math

sys.path.insert(0, "/opt/trn_rl_repo")

import numpy as np
import ml_dtypes
import concourse.bacc as bacc
import concourse.mybir as mybir
import concourse.tile as tile
from concourse.bass_utils import run_bass_kernel_spmd

P = 128
T = 2048
C = 2048
KO = C // P          # 16 contraction tiles
D = 128              # head dim
NQ = 4               # q heads per core
NK = 2               # kv heads per core
NF = NQ + NK         # 6 rope/rms feature blocks (2 k + 4 q)
FQ = NQ * D          # 512
FK = NK * D          # 256
TCH = 512            # phase-1 T-chunk
NCHUNK = T // TCH    # 4
SPAN = 512           # attention q-span
NSPAN = T // SPAN    # 4
KB = T // P          # 16 key blocks
SCALE = 1.0 / math.sqrt(D)
EPS = 1.1920929e-07

f32 = mybir.dt.float32
bf16 = mybir.dt.bfloat16

AF = mybir.ActivationFunctionType

# feature-block order in qk_rt: k0, k1, q0..q3 (K first so the last chunk's
# K is ready the moment attention starts)
FB_K = [0, 1]

def fb_of_q(h):
    return NK + h


def build():
    nc = bacc.Bacc("TRN2", target_bir_lowering=False)
    xT = nc.dram_tensor("xT", (C, T), bf16, kind="ExternalInput")
    wq = nc.dram_tensor("wq", (C, FQ), bf16, kind="ExternalInput")
    wk = nc.dram_tensor("wk", (C, FK), bf16, kind="ExternalInput")
    wv = nc.dram_tensor("wv", (C, FK), bf16, kind="ExternalInput")
    wo = nc.dram_tensor("wo", (FQ, C), bf16, kind="ExternalInput")
    cc = nc.dram_tensor("cc", (P, T), f32, kind="ExternalInput")    # [cos; cos]
    ss = nc.dram_tensor("ss", (P, T), f32, kind="ExternalInput")    # [sin; -sin]
    mask = nc.dram_tensor("mask", (P, P), bf16, kind="ExternalInput")  # tri: c>=p
    y = nc.dram_tensor("y", (T, C), bf16, kind="ExternalOutput")

    xT_r = xT.rearrange("(ko p) t -> p ko t", p=P)
    wq_r = wq.rearrange("(ko p) f -> p ko f", p=P)
    wk_r = wk.rearrange("(ko p) f -> p ko f", p=P)
    wv_r = wv.rearrange("(ko p) f -> p ko f", p=P)
    wo_r = wo.rearrange("(ko p) n -> p ko n", p=P)

    with tile.TileContext(nc) as tc:
        with tc.tile_pool(name="persist", bufs=1) as persist:
            qk_rt = persist.tile([P, NF, T], bf16, tag="qk_rt")   # roped+normed kT/qT
            v_sb = persist.tile([P, KB, FK], bf16, tag="v_sb")    # V natural [t-part, kb, feat]
            cc_sb = persist.tile([P, T], f32, tag="cc_sb")
            ss_sb = persist.tile([P, T], f32, tag="ss_sb")
            mask_sb = persist.tile([P, P], bf16, tag="mask_sb")
            ones_col = persist.tile([P, 1], bf16, tag="ones_col")
            ones_f32 = persist.tile([P, 1], f32, tag="ones_f32")
            ot_sb = persist.tile([P, NQ, T], bf16, tag="ot_sb")
            wo_sb = persist.tile([P, NQ, C], bf16, tag="wo_sb")

            eps_sb = persist.tile([1, 1], f32, tag="eps_sb")
            nc.vector.memset(eps_sb[:], EPS)
            nc.vector.memset(ones_f32[:], 1.0)
            nc.vector.tensor_copy(ones_col[:], ones_f32[:])
            nc.sync.dma_start(mask_sb[:], mask[:, :])

            # ---------------- Phase 1: QKV + RoPE + RMS norm ----------------
            with (
                tc.tile_pool(name="ph1w", bufs=1) as wpool,
                tc.tile_pool(name="ph1x", bufs=2) as xpool,
                tc.tile_pool(name="ph1t", bufs=3) as tpool,
                tc.tile_pool(name="ph1s", bufs=3) as spool,
                tc.tile_pool(name="ph1ps", bufs=3, space="PSUM") as psqk,
                tc.tile_pool(name="ph1pv", bufs=2, space="PSUM") as psv,
                tc.tile_pool(name="ph1ms", bufs=2, space="PSUM") as psms,
            ):
                wq_sb = wpool.tile([P, KO, FQ], bf16, tag="wq_sb")
                wk_sb = wpool.tile([P, KO, FK], bf16, tag="wk_sb")
                wv_sb = wpool.tile([P, KO, FK], bf16, tag="wv_sb")

                # weight source AP per feature block (fb order: k0 k1 q0..q3)
                def w_ap(fb):
                    if fb < NK:
                        return wk_sb[:, :, fb * D : (fb + 1) * D]
                    h = fb - NK
                    return wq_sb[:, :, h * D : (h + 1) * D]

                xts = [None] * NCHUNK

                def dma_chunk_part(tch, part):
                    # split the 16 per-ko DMAs into 4 batches so latency-
                    # critical swap DMAs aren't queued behind them on sync
                    t0 = tch * TCH
                    if part == 0:
                        xts[tch] = xpool.tile([P, KO, TCH], bf16, tag="xt", name="xt")
                    xt = xts[tch]
                    for ko in range(4 * part, 4 * part + 4):
                        nc.sync.dma_start(xt[:, ko, :], xT_r[:, ko, t0 : t0 + TCH])

                # startup DMA priority order: the very first matmuls need only
                # wk slice j=0 and xt chunk-0, so emit those first and push the
                # q/v weights (needed 10+ us later) behind them
                nc.sync.dma_start(wk_sb[:, :, 0:D], wk_r[:, :, 0:D])
                dma_chunk_part(0, 0)
                dma_chunk_part(0, 1)
                nc.sync.dma_start(wk_sb[:, :, D : 2 * D], wk_r[:, :, D : 2 * D])
                dma_chunk_part(0, 2)
                dma_chunk_part(0, 3)
                nc.sync.dma_start(cc_sb[:, 0:TCH], cc[:, 0:TCH])
                nc.sync.dma_start(ss_sb[:, 0:TCH], ss[:, 0:TCH])
                for h in range(NQ):
                    nc.sync.dma_start(
                        wq_sb[:, :, h * D : (h + 1) * D], wq_r[:, :, h * D : (h + 1) * D]
                    )
                nc.sync.dma_start(wv_sb[:], wv_r)

                # deferred work queue: closures emitting the rope/stats/apply
                # chain pieces, run 1-2 fb slots after the matmuls they depend on
                for tch in range(NCHUNK):
                    t0 = tch * TCH
                    xt = xts[tch]
                    stats_q = []   # (emit_stats, emit_finish) per fb
                    finish_q = []

                    def emit_fb(fb, tch=tch, t0=t0, xt=xt):
                        pqk = psqk.tile([P, TCH], f32, tag="pqk")
                        for ko in range(KO):
                            nc.tensor.matmul(
                                pqk[:], w_ap(fb)[:, ko], xt[:, ko, :],
                                start=(ko == 0), stop=(ko == KO - 1),
                            )
                        # rope chain (ACT/DMA/DVE; runs while PE does next fb)
                        raw = tpool.tile([P, TCH], f32, tag="raw")
                        nc.scalar.copy(raw[:], pqk[:])
                        swp = tpool.tile([P, TCH], f32, tag="swp")
                        nc.sync.dma_start(swp[0:64, :], raw[64:128, :])
                        nc.sync.dma_start(swp[64:128, :], raw[0:64, :])
                        seg = qk_rt[:, fb, t0 : t0 + TCH]
                        tmpa = tpool.tile([P, TCH], f32, tag="tmpa")
                        nc.vector.tensor_mul(tmpa[:], pqk[:], cc_sb[:, t0 : t0 + TCH])
                        tmpb = tpool.tile([P, TCH], f32, tag="tmpb")
                        nc.vector.tensor_mul(tmpb[:], swp[:], ss_sb[:, t0 : t0 + TCH])
                        nc.vector.tensor_add(seg, tmpa[:], tmpb[:])
                        sq = spool.tile([P, TCH], bf16, tag="sq")
                        nc.vector.tensor_mul(sq[:], seg, seg)

                        def emit_stats(fb=fb, sq=sq):
                            pms = psms.tile([1, TCH], f32, tag="pms")
                            nc.tensor.matmul(
                                pms[:], ones_col[:], sq[:], start=True, stop=True
                            )
                            # rms = sqrt(ms/D + eps) on ACT (same table all phase)
                            rms = spool.tile([1, TCH], f32, tag="rms")
                            nc.scalar.activation(
                                rms[:], pms[:], AF.Sqrt, bias=eps_sb[0:1, :],
                                scale=1.0 / D,
                            )
                            return rms

                        def emit_finish(rms, fb=fb, seg=seg):
                            rstd = spool.tile([1, TCH], f32, tag="rstd")
                            nc.vector.reciprocal_approx_fast(rstd[:], rms[:])
                            rb = tpool.tile([P, TCH], f32, tag="rb")
                            nc.gpsimd.partition_broadcast(rb[:], rstd[:])
                            nc.vector.tensor_mul(seg, seg, rb[:])

                        stats_q.append(emit_stats)
                        finish_q.append(emit_finish)

                    # V blocks, natural layout: stationary = x block, moving = wv
                    def emit_v(tb, tch=tch, xt=xt):
                        pv = psv.tile([P, FK], f32, tag="pv")
                        for ko in range(KO):
                            nc.tensor.matmul(
                                pv[:], xt[:, ko, tb * P : (tb + 1) * P], wv_sb[:, ko, :],
                                start=(ko == 0), stop=(ko == KO - 1),
                            )
                        nc.vector.tensor_copy(v_sb[:, tch * (TCH // P) + tb, :], pv[:])

                    # PE emission order for this chunk: stats matmul for fb is
                    # emitted 2 fb-slots later, the rstd/apply chain 4 slots
                    # later, so the PE never waits on the DVE/ACT chains.
                    rms_tiles = [None] * NF
                    for fb in range(NF):
                        emit_fb(fb)
                        if 1 <= fb <= 4 and tch + 1 < NCHUNK:
                            dma_chunk_part(tch + 1, fb - 1)
                        if fb == 5 and tch + 1 < NCHUNK:
                            t1 = (tch + 1) * TCH
                            nc.sync.dma_start(cc_sb[:, t1 : t1 + TCH], cc[:, t1 : t1 + TCH])
                            nc.sync.dma_start(ss_sb[:, t1 : t1 + TCH], ss[:, t1 : t1 + TCH])
                        if fb >= 2:
                            rms_tiles[fb - 2] = stats_q[fb - 2]()
                        if fb >= 4:
                            finish_q[fb - 4](rms_tiles[fb - 4])
                    for tb in range(TCH // P):
                        emit_v(tb)
                        if tb < 2:
                            rms_tiles[NF - 2 + tb] = stats_q[NF - 2 + tb]()
                        finish_q[NF - 4 + tb](rms_tiles[NF - 4 + tb])

                nc.sync.dma_start(wo_sb[:], wo_r)

            # ---------------- Phase 2: attention + output projection ----------------
            pending_proj = []

            def make_proj_emitter(psy, ypool):
                def emit_proj_group():
                    tb, nch = pending_proj.pop(0)
                    yps = psy.tile([P, 512], f32, tag="yps", name="yps")
                    for h in range(NQ):
                        nc.tensor.matmul(
                            yps[:],
                            ot_sb[:, h, tb * P : (tb + 1) * P],
                            wo_sb[:, h, nch * 512 : (nch + 1) * 512],
                            start=(h == 0), stop=(h == NQ - 1),
                        )
                    ysb = ypool.tile([P, 512], bf16, tag="ysb", name="ysb")
                    nc.vector.tensor_copy(ysb[:], yps[:])
                    nc.sync.dma_start(
                        y[tb * P : (tb + 1) * P, nch * 512 : (nch + 1) * 512], ysb[:]
                    )
                return emit_proj_group

            with (
                tc.tile_pool(name="at_pt", bufs=6) as ptpool,
                tc.tile_pool(name="at_sb", bufs=3) as asb,
                tc.tile_pool(name="at_rb", bufs=2) as rbpool,
                tc.tile_pool(name="at_y", bufs=3) as ypool,
                tc.tile_pool(name="at_st", bufs=2, space="PSUM") as psst,
                tc.tile_pool(name="at_ot", bufs=2, space="PSUM") as psot,
                tc.tile_pool(name="at_sm", bufs=1, space="PSUM") as pssm,
                tc.tile_pool(name="at_yp", bufs=1, space="PSUM") as psy,
            ):
                emit_proj_group = make_proj_emitter(psy, ypool)

                for s in range(NSPAN):
                    q0 = s * SPAN
                    nkb = 4 * (s + 1)
                    npair = nkb // 2
                    span_iters = NQ * npair
                    span_pending = len(pending_proj)
                    it = 0

                    def off_of(kb, s=s):
                        return P * (kb - 4 * s) if kb >= 4 * s else 0

                    for h in range(NQ):
                        j = h // 2
                        q_ap = qk_rt[:, fb_of_q(h), q0 : q0 + SPAN]
                        ot_ps = psot.tile([P, SPAN], f32, tag="ot_ps")
                        sum_ps = pssm.tile([1, SPAN], f32, tag="sum_ps")

                        def emit_avsum(pairs, h=h, j=j, nkb=nkb,
                                       ot_ps=ot_ps, sum_ps=sum_ps):
                            # group by destination bank: accumulate-mode bank
                            # switches cost ~80ns of PE turnaround each, so
                            # emit all AVs then all sums for a batch of pairs
                            kbs = [(kb, pt) for pa in pairs
                                   for kb, pt in ((pa[0], pa[1]), (pa[2], pa[3]))]
                            for kb, pt_ap in kbs:
                                nc.tensor.matmul(
                                    ot_ps[:, off_of(kb):],
                                    v_sb[:, kb, j * D : (j + 1) * D], pt_ap,
                                    start=(kb == 0), stop=(kb == nkb - 1),
                                    skip_group_check=True,
                                )
                            for kb, pt_ap in kbs:
                                nc.tensor.matmul(
                                    sum_ps[:, off_of(kb):], ones_col[:], pt_ap,
                                    start=(kb == 0), stop=(kb == nkb - 1),
                                    skip_group_check=True,
                                )

                        # AV/sum run TWO kb-pairs behind the scores so the
                        # exp chain (plus its two semaphore hops) never gates
                        # the PE
                        pipe = []  # queued (kb_a, pt_a_ap, kb_b, pt_b_ap)
                        for i in range(npair):
                            ka, kbb = 2 * i, 2 * i + 1
                            offa, offb = off_of(ka), off_of(kbb)
                            diag = kbb >= 4 * s
                            st2 = psst.tile([P, 2, SPAN], f32, tag="st2")
                            nc.tensor.matmul(
                                st2[:, 0, offa:],
                                qk_rt[:, j, ka * P : (ka + 1) * P],
                                q_ap[:, offa:],
                                start=True, stop=True,
                            )
                            nc.tensor.matmul(
                                st2[:, 1, offb:],
                                qk_rt[:, j, kbb * P : (kbb + 1) * P],
                                q_ap[:, offb:],
                                start=True, stop=True,
                            )
                            # PE filler while exp runs: AV/sums from 2+ pairs
                            # ago, batched two pairs at a time
                            if len(pipe) >= 3:
                                emit_avsum([pipe.pop(0), pipe.pop(0)])
                            # spread the previous span's projection groups
                            # evenly over this span's iterations (PE filler
                            # that overlaps the exp chain)
                            n_pop = ((it + 1) * span_pending) // span_iters - (
                                it * span_pending
                            ) // span_iters
                            for _ in range(min(n_pop, len(pending_proj))):
                                emit_proj_group()
                            it += 1
                            # exp (+ mask on diagonal blocks)
                            pt2 = ptpool.tile([P, 2, SPAN], bf16, tag="pt2")
                            if diag:
                                nc.scalar.activation(
                                    pt2[:, 0, offa:], st2[:, 0, offa:], AF.Exp,
                                    scale=SCALE,
                                )
                                nc.scalar.activation(
                                    pt2[:, 1, offb:], st2[:, 1, offb:], AF.Exp,
                                    scale=SCALE,
                                )
                                if ka >= 4 * s:
                                    nc.vector.tensor_mul(
                                        pt2[:, 0, offa : offa + P],
                                        pt2[:, 0, offa : offa + P],
                                        mask_sb[:],
                                    )
                                nc.vector.tensor_mul(
                                    pt2[:, 1, offb : offb + P],
                                    pt2[:, 1, offb : offb + P],
                                    mask_sb[:],
                                )
                            else:
                                nc.scalar.activation(
                                    pt2[:, :, :], st2[:, :, :], AF.Exp, scale=SCALE
                                )
                            pipe.append((ka, pt2[:, 0, offa:], kbb, pt2[:, 1, offb:]))
                        while pipe:
                            emit_avsum([pipe.pop(0) for _ in range(min(2, len(pipe)))])

                        # normalization tail: 1/sums on DVE, bcast on GpSimd
                        rec = asb.tile([1, SPAN], f32, tag="rec")
                        nc.vector.reciprocal_approx_fast(rec[:], sum_ps[:])
                        rb = rbpool.tile([P, SPAN], f32, tag="rb2")
                        nc.gpsimd.partition_broadcast(rb[:], rec[:])
                        nc.vector.tensor_mul(
                            ot_sb[:, h, q0 : q0 + SPAN], ot_ps[:], rb[:]
                        )

                    # queue this span's projection; emitted inside span s+1's
                    # loop as PE filler (span 3's drains below)
                    for tb in range(4 * s, 4 * s + 4):
                        for nch in range(C // 512):
                            pending_proj.append((tb, nch))

            # span 3's projection: own PSUM pool (3 banks, freed by the
            # attention pools closing) so the groups pipeline through the
            # PSUM->SBUF copies without serializing
            with (
                tc.tile_pool(name="tl_y", bufs=3) as ypool2,
                tc.tile_pool(name="tl_yp", bufs=3, space="PSUM") as psy2,
            ):
                emit_tail_group = make_proj_emitter(psy2, ypool2)
                while pending_proj:
                    emit_tail_group()
    nc.compile()
    return nc


_NC_CACHE = None


def _get_nc():
    global _NC_CACHE
    if _NC_CACHE is None:
        _NC_CACHE = build()
    return _NC_CACHE


def _host_inputs(x, cos, sin, wq, wk, wv, wo):
    """Build the 8 per-core input maps."""
    bft = ml_dtypes.bfloat16
    cosT = np.ascontiguousarray(cos[0, :, 0, :].T).astype(np.float32)  # (64, T)
    sinT = np.ascontiguousarray(sin[0, :, 0, :].T).astype(np.float32)
    cc = np.concatenate([cosT, cosT], axis=0)          # (128, T)
    ss = np.concatenate([sinT, -sinT], axis=0)
    # mask[p, c] = 1 if c >= p (upper-tri incl diag): within a diagonal
    # 128-block, q-offset c sees key-offset p iff c >= p
    mask = (np.arange(P)[None, :] >= np.arange(P)[:, None]).astype(bft)

    xTs = [np.ascontiguousarray(x[b].T).astype(bft) for b in range(2)]
    wq16 = wq.astype(bft)
    wk16 = wk.astype(bft)
    wv16 = wv.astype(bft)
    wo16 = wo.astype(bft)
    in_maps = []
    for c in range(8):
        b, tp = divmod(c, 4)
        in_maps.append(
            {
                "xT": xTs[b],
                "wq": np.ascontiguousarray(wq16[:, tp * FQ : (tp + 1) * FQ]),
                "wk": np.ascontiguousarray(wk16[:, tp * FK : (tp + 1) * FK]),
                "wv": np.ascontiguousarray(wv16[:, tp * FK : (tp + 1) * FK]),
                "wo": np.ascontiguousarray(wo16[tp * FQ : (tp + 1) * FQ, :]),
                "cc": cc,
                "ss": ss,
                "mask": mask,
            }
        )
    return in_maps


def kernel(x, cos, sin, wq, wk, wv, wo, trace=False):
    x = np.asarray(x, dtype=np.float32)
    cos = np.asarray(cos, dtype=np.float32)
    sin = np.asarray(sin, dtype=np.float32)
    wq = np.asarray(wq, dtype=np.float32)
    wk = np.asarray(wk, dtype=np.float32)
    wv = np.asarray(wv, dtype=np.float32)
    wo = np.asarray(wo, dtype=np.float32)

    nc = _get_nc()
    in_maps = _host_inputs(x, cos, sin, wq, wk, wv, wo)
    res = run_bass_kernel_spmd(nc, in_maps, core_ids=list(range(8)), trace=trace)
    out = np.zeros((2, T, C), dtype=np.float32)
    for c in range(8):
        b = c // 4
        out[b] += res.results[c]["y"].astype(np.float32)
    if trace:
        return out, res
    return out


# revision 28
# speedup vs baseline: 1.4313x; 1.0012x over previous
"""Causal self-attention (RoPE + QK-RMSNorm, GQA 16q/8kv) Trainium2 Bass kernel.

Sharding: 8 cores = 2 batch x 4 tensor-parallel. Core c handles batch b=c//4 and
q-heads [4*tp, 4*tp+4), kv-heads [2*tp, 2*tp+2) where tp=c%4. Each core returns a
partial (T, C) output = O_heads @ wo[rows of its heads]; host sums the 4 partials
per batch (the "all-reduce after c_proj").

Perf notes vs the first working version:
- The scalar engine only ever runs Sqrt (phase 1) and Exp (attention) plus Copy,
  so there is no per-iteration activation-table reload.
- The PE stream is ordered so every cross-engine dependency has >1us of queued
  PE work in front of it: RMS stats matmuls are deferred one feature block,
  attention AV/sum matmuls run one kb-pair behind the score matmuls, and the
  output projection of span s-1 is interleaved into span s's loop as filler.
  This keeps the tensor engine continuously busy, which also keeps it at the
  2.4 GHz p-state (it drops to 1.2 GHz for 3us after every idle gap).
- V is produced directly in [t, feat] layout (x-block as stationary operand),
  removing the PE transposes.
- Softmax normalization: row sums via ones-column matmuls (PSUM), reciprocal on
  DVE (reciprocal_approx_fast), broadcast across partitions on GpSimd.
- Causal diagonal blocks are trimmed: score/exp/AV/sum only touch q-columns
  that can be live for that key block.
"""
import sys
import math

sys.path.insert(0, "/opt/trn_rl_repo")

import numpy as np
import ml_dtypes
import concourse.bacc as bacc
import concourse.mybir as mybir
import concourse.tile as tile
from concourse.bass_utils import run_bass_kernel_spmd

P = 128
T = 2048
C = 2048
KO = C // P          # 16 contraction tiles
D = 128              # head dim
NQ = 4               # q heads per core
NK = 2               # kv heads per core
NF = NQ + NK         # 6 rope/rms feature blocks (2 k + 4 q)
FQ = NQ * D          # 512
FK = NK * D          # 256
TCH = 512            # phase-1 T-chunk
NCHUNK = T // TCH    # 4
SPAN = 512           # attention q-span
NSPAN = T // SPAN    # 4
KB = T // P          # 16 key blocks
SCALE = 1.0 / math.sqrt(D)
EPS = 1.1920929e-07

f32 = mybir.dt.float32
bf16 = mybir.dt.bfloat16

AF = mybir.ActivationFunctionType

# feature-block order in qk_rt: k0, k1, q0..q3 (K first so the last chunk's
# K is ready the moment attention starts)
FB_K = [0, 1]

def fb_of_q(h):
    return NK + h


def build():
    nc = bacc.Bacc("TRN2", target_bir_lowering=False)
    xT = nc.dram_tensor("xT", (C, T), bf16, kind="ExternalInput")
    wq = nc.dram_tensor("wq", (C, FQ), bf16, kind="ExternalInput")
    wk = nc.dram_tensor("wk", (C, FK), bf16, kind="ExternalInput")
    wv = nc.dram_tensor("wv", (C, FK), bf16, kind="ExternalInput")
    wo = nc.dram_tensor("wo", (FQ, C), bf16, kind="ExternalInput")
    cc = nc.dram_tensor("cc", (P, T), f32, kind="ExternalInput")    # [cos; cos]
    ss = nc.dram_tensor("ss", (P, T), f32, kind="ExternalInput")    # [sin; -sin]
    mask = nc.dram_tensor("mask", (P, P), bf16, kind="ExternalInput")  # tri: c>=p
    y = nc.dram_tensor("y", (T, C), bf16, kind="ExternalOutput")

    xT_r = xT.rearrange("(ko p) t -> p ko t", p=P)
    wq_r = wq.rearrange("(ko p) f -> p ko f", p=P)
    wk_r = wk.rearrange("(ko p) f -> p ko f", p=P)
    wv_r = wv.rearrange("(ko p) f -> p ko f", p=P)
    wo_r = wo.rearrange("(ko p) n -> p ko n", p=P)

    with tile.TileContext(nc) as tc:
        with tc.tile_pool(name="persist", bufs=1) as persist:
            qk_rt = persist.tile([P, NF, T], bf16, tag="qk_rt")   # roped+normed kT/qT
            v_sb = persist.tile([P, KB, FK], bf16, tag="v_sb")    # V natural [t-part, kb, feat]
            cc_sb = persist.tile([P, T], f32, tag="cc_sb")
            ss_sb = persist.tile([P, T], f32, tag="ss_sb")
            mask_sb = persist.tile([P, P], bf16, tag="mask_sb")
            ones_col = persist.tile([P, 1], bf16, tag="ones_col")
            ones_f32 = persist.tile([P, 1], f32, tag="ones_f32")
            ot_sb = persist.tile([P, NQ, T], bf16, tag="ot_sb")
            wo_sb = persist.tile([P, NQ, C], bf16, tag="wo_sb")

            eps_sb = persist.tile([1, 1], f32, tag="eps_sb")
            nc.vector.memset(eps_sb[:], EPS)
            nc.vector.memset(ones_f32[:], 1.0)
            nc.vector.tensor_copy(ones_col[:], ones_f32[:])
            nc.sync.dma_start(mask_sb[:], mask[:, :])

            # ---------------- Phase 1: QKV + RoPE + RMS norm ----------------
            with (
                tc.tile_pool(name="ph1w", bufs=1) as wpool,
                tc.tile_pool(name="ph1x", bufs=2) as xpool,
                tc.tile_pool(name="ph1t", bufs=3) as tpool,
                tc.tile_pool(name="ph1s", bufs=3) as spool,
                tc.tile_pool(name="ph1ps", bufs=3, space="PSUM") as psqk,
                tc.tile_pool(name="ph1pv", bufs=2, space="PSUM") as psv,
                tc.tile_pool(name="ph1ms", bufs=2, space="PSUM") as psms,
            ):
                wq_sb = wpool.tile([P, KO, FQ], bf16, tag="wq_sb")
                wk_sb = wpool.tile([P, KO, FK], bf16, tag="wk_sb")
                wv_sb = wpool.tile([P, KO, FK], bf16, tag="wv_sb")

                # weight source AP per feature block (fb order: k0 k1 q0..q3)
                def w_ap(fb):
                    if fb < NK:
                        return wk_sb[:, :, fb * D : (fb + 1) * D]
                    h = fb - NK
                    return wq_sb[:, :, h * D : (h + 1) * D]

                xts = [None] * NCHUNK

                def dma_chunk_part(tch, part):
                    # split the 16 per-ko DMAs into 4 batches so latency-
                    # critical swap DMAs aren't queued behind them on sync
                    t0 = tch * TCH
                    if part == 0:
                        xts[tch] = xpool.tile([P, KO, TCH], bf16, tag="xt", name="xt")
                    xt = xts[tch]
                    for ko in range(4 * part, 4 * part + 4):
                        nc.sync.dma_start(xt[:, ko, :], xT_r[:, ko, t0 : t0 + TCH])

                # startup DMA priority order: the very first matmuls need only
                # wk slice j=0 and xt chunk-0, so emit those first and push the
                # q/v weights (needed 10+ us later) behind them
                nc.sync.dma_start(wk_sb[:], wk_r)
                for part in range(4):
                    dma_chunk_part(0, part)
                nc.sync.dma_start(cc_sb[:, 0:TCH], cc[:, 0:TCH])
                nc.sync.dma_start(ss_sb[:, 0:TCH], ss[:, 0:TCH])
                for h in range(NQ):
                    nc.sync.dma_start(
                        wq_sb[:, :, h * D : (h + 1) * D], wq_r[:, :, h * D : (h + 1) * D]
                    )
                nc.sync.dma_start(wv_sb[:], wv_r)

                # deferred work queue: closures emitting the rope/stats/apply
                # chain pieces, run 1-2 fb slots after the matmuls they depend on
                for tch in range(NCHUNK):
                    t0 = tch * TCH
                    xt = xts[tch]
                    stats_q = []   # (emit_stats, emit_finish) per fb
                    finish_q = []

                    def emit_fb(fb, tch=tch, t0=t0, xt=xt):
                        pqk = psqk.tile([P, TCH], f32, tag="pqk")
                        for ko in range(KO):
                            nc.tensor.matmul(
                                pqk[:], w_ap(fb)[:, ko], xt[:, ko, :],
                                start=(ko == 0), stop=(ko == KO - 1),
                            )
                        # rope chain (ACT/DMA/DVE; runs while PE does next fb)
                        raw = tpool.tile([P, TCH], f32, tag="raw")
                        nc.scalar.copy(raw[:], pqk[:])
                        swp = tpool.tile([P, TCH], f32, tag="swp")
                        nc.sync.dma_start(swp[0:64, :], raw[64:128, :])
                        nc.sync.dma_start(swp[64:128, :], raw[0:64, :])
                        seg = qk_rt[:, fb, t0 : t0 + TCH]
                        tmpa = tpool.tile([P, TCH], f32, tag="tmpa")
                        nc.vector.tensor_mul(tmpa[:], pqk[:], cc_sb[:, t0 : t0 + TCH])
                        tmpb = tpool.tile([P, TCH], f32, tag="tmpb")
                        nc.vector.tensor_mul(tmpb[:], swp[:], ss_sb[:, t0 : t0 + TCH])
                        nc.vector.tensor_add(seg, tmpa[:], tmpb[:])
                        sq = spool.tile([P, TCH], bf16, tag="sq")
                        nc.vector.tensor_mul(sq[:], seg, seg)

                        def emit_stats(fb=fb, sq=sq):
                            pms = psms.tile([1, TCH], f32, tag="pms")
                            nc.tensor.matmul(
                                pms[:], ones_col[:], sq[:], start=True, stop=True
                            )
                            # rms = sqrt(ms/D + eps) on ACT (same table all phase)
                            rms = spool.tile([1, TCH], f32, tag="rms")
                            nc.scalar.activation(
                                rms[:], pms[:], AF.Sqrt, bias=eps_sb[0:1, :],
                                scale=1.0 / D,
                            )
                            return rms

                        def emit_finish(rms, fb=fb, seg=seg):
                            rstd = spool.tile([1, TCH], f32, tag="rstd")
                            nc.vector.reciprocal_approx_fast(rstd[:], rms[:])
                            rb = tpool.tile([P, TCH], f32, tag="rb")
                            nc.gpsimd.partition_broadcast(rb[:], rstd[:])
                            nc.vector.tensor_mul(seg, seg, rb[:])

                        stats_q.append(emit_stats)
                        finish_q.append(emit_finish)

                    # V blocks, natural layout: stationary = x block, moving = wv
                    def emit_v(tb, tch=tch, xt=xt):
                        pv = psv.tile([P, FK], f32, tag="pv")
                        for ko in range(KO):
                            nc.tensor.matmul(
                                pv[:], xt[:, ko, tb * P : (tb + 1) * P], wv_sb[:, ko, :],
                                start=(ko == 0), stop=(ko == KO - 1),
                            )
                        nc.vector.tensor_copy(v_sb[:, tch * (TCH // P) + tb, :], pv[:])

                    # PE emission order for this chunk: stats matmul for fb is
                    # emitted 2 fb-slots later, the rstd/apply chain 4 slots
                    # later, so the PE never waits on the DVE/ACT chains.
                    rms_tiles = [None] * NF
                    for fb in range(NF):
                        emit_fb(fb)
                        if 1 <= fb <= 4 and tch + 1 < NCHUNK:
                            dma_chunk_part(tch + 1, fb - 1)
                        if fb == 5 and tch + 1 < NCHUNK:
                            t1 = (tch + 1) * TCH
                            nc.sync.dma_start(cc_sb[:, t1 : t1 + TCH], cc[:, t1 : t1 + TCH])
                            nc.sync.dma_start(ss_sb[:, t1 : t1 + TCH], ss[:, t1 : t1 + TCH])
                        if fb >= 2:
                            rms_tiles[fb - 2] = stats_q[fb - 2]()
                        if fb >= 4:
                            finish_q[fb - 4](rms_tiles[fb - 4])
                    for tb in range(TCH // P):
                        emit_v(tb)
                        if tb < 2:
                            rms_tiles[NF - 2 + tb] = stats_q[NF - 2 + tb]()
                        finish_q[NF - 4 + tb](rms_tiles[NF - 4 + tb])

                nc.sync.dma_start(wo_sb[:], wo_r)

            # ---------------- Phase 2: attention + output projection ----------------
            pending_proj = []

            def make_proj_emitter(psy, ypool):
                def emit_proj_group():
                    tb, nch = pending_proj.pop(0)
                    yps = psy.tile([P, 512], f32, tag="yps", name="yps")
                    for h in range(NQ):
                        nc.tensor.matmul(
                            yps[:],
                            ot_sb[:, h, tb * P : (tb + 1) * P],
                            wo_sb[:, h, nch * 512 : (nch + 1) * 512],
                            start=(h == 0), stop=(h == NQ - 1),
                        )
                    ysb = ypool.tile([P, 512], bf16, tag="ysb", name="ysb")
                    nc.vector.tensor_copy(ysb[:], yps[:])
                    nc.sync.dma_start(
                        y[tb * P : (tb + 1) * P, nch * 512 : (nch + 1) * 512], ysb[:]
                    )
                return emit_proj_group

            with (
                tc.tile_pool(name="at_pt", bufs=6) as ptpool,
                tc.tile_pool(name="at_sb", bufs=3) as asb,
                tc.tile_pool(name="at_rb", bufs=2) as rbpool,
                tc.tile_pool(name="at_y", bufs=3) as ypool,
                tc.tile_pool(name="at_st", bufs=2, space="PSUM") as psst,
                tc.tile_pool(name="at_ot", bufs=2, space="PSUM") as psot,
                tc.tile_pool(name="at_sm", bufs=1, space="PSUM") as pssm,
                tc.tile_pool(name="at_yp", bufs=1, space="PSUM") as psy,
            ):
                emit_proj_group = make_proj_emitter(psy, ypool)

                for s in range(NSPAN):
                    q0 = s * SPAN
                    nkb = 4 * (s + 1)
                    npair = nkb // 2
                    span_iters = NQ * npair
                    span_pending = len(pending_proj)
                    it = 0

                    def off_of(kb, s=s):
                        return P * (kb - 4 * s) if kb >= 4 * s else 0

                    for h in range(NQ):
                        j = h // 2
                        q_ap = qk_rt[:, fb_of_q(h), q0 : q0 + SPAN]
                        ot_ps = psot.tile([P, SPAN], f32, tag="ot_ps")
                        sum_ps = pssm.tile([1, SPAN], f32, tag="sum_ps")

                        def emit_avsum(pairs, h=h, j=j, nkb=nkb,
                                       ot_ps=ot_ps, sum_ps=sum_ps):
                            # group by destination bank: accumulate-mode bank
                            # switches cost ~80ns of PE turnaround each, so
                            # emit all AVs then all sums for a batch of pairs
                            kbs = [(kb, pt) for pa in pairs
                                   for kb, pt in ((pa[0], pa[1]), (pa[2], pa[3]))]
                            for kb, pt_ap in kbs:
                                nc.tensor.matmul(
                                    ot_ps[:, off_of(kb):],
                                    v_sb[:, kb, j * D : (j + 1) * D], pt_ap,
                                    start=(kb == 0), stop=(kb == nkb - 1),
                                    skip_group_check=True,
                                )
                            for kb, pt_ap in kbs:
                                nc.tensor.matmul(
                                    sum_ps[:, off_of(kb):], ones_col[:], pt_ap,
                                    start=(kb == 0), stop=(kb == nkb - 1),
                                    skip_group_check=True,
                                )

                        # AV/sum run TWO kb-pairs behind the scores so the
                        # exp chain (plus its two semaphore hops) never gates
                        # the PE
                        pipe = []  # queued (kb_a, pt_a_ap, kb_b, pt_b_ap)
                        for i in range(npair):
                            ka, kbb = 2 * i, 2 * i + 1
                            offa, offb = off_of(ka), off_of(kbb)
                            diag = kbb >= 4 * s
                            st2 = psst.tile([P, 2, SPAN], f32, tag="st2")
                            nc.tensor.matmul(
                                st2[:, 0, offa:],
                                qk_rt[:, j, ka * P : (ka + 1) * P],
                                q_ap[:, offa:],
                                start=True, stop=True,
                            )
                            nc.tensor.matmul(
                                st2[:, 1, offb:],
                                qk_rt[:, j, kbb * P : (kbb + 1) * P],
                                q_ap[:, offb:],
                                start=True, stop=True,
                            )
                            # PE filler while exp runs: AV/sums from 2+ pairs
                            # ago, batched two pairs at a time
                            if len(pipe) >= 3:
                                emit_avsum([pipe.pop(0), pipe.pop(0)])
                            # spread the previous span's projection groups
                            # evenly over this span's iterations (PE filler
                            # that overlaps the exp chain)
                            n_pop = ((it + 1) * span_pending) // span_iters - (
                                it * span_pending
                            ) // span_iters
                            for _ in range(min(n_pop, len(pending_proj))):
                                emit_proj_group()
                            it += 1
                            # exp (+ mask on diagonal blocks)
                            pt2 = ptpool.tile([P, 2, SPAN], bf16, tag="pt2")
                            if diag:
                                nc.scalar.activation(
                                    pt2[:, 0, offa:], st2[:, 0, offa:], AF.Exp,
                                    scale=SCALE,
                                )
                                nc.scalar.activation(
                                    pt2[:, 1, offb:], st2[:, 1, offb:], AF.Exp,
                                    scale=SCALE,
                                )
                                if ka >= 4 * s:
                                    nc.vector.tensor_mul(
                                        pt2[:, 0, offa : offa + P],
                                        pt2[:, 0, offa : offa + P],
                                        mask_sb[:],
                                    )
                                nc.vector.tensor_mul(
                                    pt2[:, 1, offb : offb + P],
                                    pt2[:, 1, offb : offb + P],
                                    mask_sb[:],
                                )
                            else:
                                nc.scalar.activation(
                                    pt2[:, :, :], st2[:, :, :], AF.Exp, scale=SCALE
                                )
                            pipe.append((ka, pt2[:, 0, offa:], kbb, pt2[:, 1, offb:]))
                        while pipe:
                            emit_avsum([pipe.pop(0) for _ in range(min(2, len(pipe)))])

                        # normalization tail: 1/sums on DVE, bcast on GpSimd
                        rec = asb.tile([1, SPAN], f32, tag="rec")
                        nc.vector.reciprocal_approx_fast(rec[:], sum_ps[:])
                        rb = rbpool.tile([P, SPAN], f32, tag="rb2")
                        nc.gpsimd.partition_broadcast(rb[:], rec[:])
                        nc.vector.tensor_mul(
                            ot_sb[:, h, q0 : q0 + SPAN], ot_ps[:], rb[:]
                        )

                    # queue this span's projection; emitted inside span s+1's
                    # loop as PE filler (span 3's drains below)
                    for tb in range(4 * s, 4 * s + 4):
                        for nch in range(C // 512):
                            pending_proj.append((tb, nch))

            # span 3's projection: own PSUM pool (3 banks, freed by the
            # attention pools closing) so the groups pipeline through the
            # PSUM->SBUF copies without serializing
            with (
                tc.tile_pool(name="tl_y", bufs=3) as ypool2,
                tc.tile_pool(name="tl_yp", bufs=3, space="PSUM") as psy2,
            ):
                emit_tail_group = make_proj_emitter(psy2, ypool2)
                while pending_proj:
                    emit_tail_group()
    nc.compile()
    return nc


_NC_CACHE = None


def _get_nc():
    global _NC_CACHE
    if _NC_CACHE is None:
        _NC_CACHE = build()
    return _NC_CACHE


def _host_inputs(x, cos, sin, wq, wk, wv, wo):
    """Build the 8 per-core input maps."""
    bft = ml_dtypes.bfloat16
    cosT = np.ascontiguousarray(cos[0, :, 0, :].T).astype(np.float32)  # (64, T)
    sinT = np.ascontiguousarray(sin[0, :, 0, :].T).astype(np.float32)
    cc = np.concatenate([cosT, cosT], axis=0)          # (128, T)
    ss = np.concatenate([sinT, -sinT], axis=0)
    # mask[p, c] = 1 if c >= p (upper-tri incl diag): within a diagonal
    # 128-block, q-offset c sees key-offset p iff c >= p
    mask = (np.arange(P)[None, :] >= np.arange(P)[:, None]).astype(bft)

    xTs = [np.ascontiguousarray(x[b].T).astype(bft) for b in range(2)]
    wq16 = wq.astype(bft)
    wk16 = wk.astype(bft)
    wv16 = wv.astype(bft)
    wo16 = wo.astype(bft)
    in_maps = []
    for c in range(8):
        b, tp = divmod(c, 4)
        in_maps.append(
            {
                "xT": xTs[b],
                "wq": np.ascontiguousarray(wq16[:, tp * FQ : (tp + 1) * FQ]),
                "wk": np.ascontiguousarray(wk16[:, tp * FK : (tp + 1) * FK]),
                "wv": np.ascontiguousarray(wv16[:, tp * FK : (tp + 1) * FK]),
                "wo": np.ascontiguousarray(wo16[tp * FQ : (tp + 1) * FQ, :]),
                "cc": cc,
                "ss": ss,
                "mask": mask,
            }
        )
    return in_maps


def kernel(x, cos, sin, wq, wk, wv, wo, trace=False):
    x = np.asarray(x, dtype=np.float32)
    cos = np.asarray(cos, dtype=np.float32)
    sin = np.asarray(sin, dtype=np.float32)
    wq = np.asarray(wq, dtype=np.float32)
    wk = np.asarray(wk, dtype=np.float32)
    wv = np.asarray(wv, dtype=np.float32)
    wo = np.asarray(wo, dtype=np.float32)

    nc = _get_nc()
    in_maps = _host_inputs(x, cos, sin, wq, wk, wv, wo)
    res = run_bass_kernel_spmd(nc, in_maps, core_ids=list(range(8)), trace=trace)
    out = np.zeros((2, T, C), dtype=np.float32)
    for c in range(8):
        b = c // 4
        out[b] += res.results[c]["y"].astype(np.float32)
    if trace:
        return out, res
    return out
